# revision 1
# baseline (speedup 1.0000x reference)
"""CrossScaleAttention Trainium2 kernel: 8-core SPMD via bass/tile.

Sharding: core (s, py) = (core//2, core%2): sample s = core//2, output row
parity py. Each core computes full attention for its sample and the deconv
for its output-row parity. Host prepares small gather tensors (q_col, kpT,
rnorm, ap taps — <0.1% of FLOPs); device does scores matmuls, softmax
(exp/Z/normalize) and the conv-transpose matmuls in fp32r.
"""
import sys, types
sys.path.insert(0, "/opt/trn_rl_repo")
import numpy as np
from contextlib import ExitStack

# NTFF profile hook shim (image's antenv lacks axon_hooks)
try:
    import trn_agent_boot.trn_boot as _tb
    _hook = _tb._ntff_profile_via_ctypes('/opt/axon/libaxon_pjrt.so')
    _m = types.ModuleType("antenv.axon_hooks")
    _m.get_axon_ntff_profile_hook = lambda: _hook
    _m.set_axon_ntff_profile_hook = lambda h: None
    sys.modules["antenv.axon_hooks"] = _m
except Exception:
    pass

import concourse.bass as bass
import concourse.tile as tile
import concourse.mybir as mybir
from concourse import bacc
from concourse.bass_utils import run_bass_kernel_spmd

F32 = mybir.dt.float32
F32R = mybir.dt.float32r
AF = mybir.ActivationFunctionType

C, Cr, B, H, W, L = 64, 16, 4, 96, 96, 2304
NCH = 18           # l-chunks of 128
ST_A = 6           # a-rows (output row-pairs) per stripe
RWS = ST_A + 2     # att i-rows buffered per stripe
NST = 96 // ST_A   # stripes

last_exec_time_ns = None

_cache = {}


def _build_program():
    nc = bacc.Bacc("TRN2", target_bir_lowering=False, debug=False, num_devices=8)
    qA_d = nc.dram_tensor("qcolA", [128, H * W], F32R, kind="ExternalInput").ap()
    qB_d = nc.dram_tensor("qcolB", [16, H * W], F32R, kind="ExternalInput").ap()
    kA_d = nc.dram_tensor("kpTA", [128, L], F32R, kind="ExternalInput").ap()
    kB_d = nc.dram_tensor("kpTB", [16, L], F32R, kind="ExternalInput").ap()
    rn_d = nc.dram_tensor("rnt", [128, NCH], F32, kind="ExternalInput").ap()
    ap_d = nc.dram_tensor("ap", [9 * L, 128], F32R, kind="ExternalInput").ap()
    oh_d = nc.dram_tensor("oh", [64, 96 * 192], F32, kind="ExternalOutput").ap()

    with tile.TileContext(nc) as tc:
        with ExitStack() as ctx:
            pm = ctx.enter_context(tc.tile_pool(name="main", bufs=1))
            pq = ctx.enter_context(tc.tile_pool(name="q", bufs=2))
            pob = ctx.enter_context(tc.tile_pool(name="ob", bufs=3))
            prz = ctx.enter_context(tc.tile_pool(name="rz", bufs=2))
            pps = ctx.enter_context(tc.tile_pool(name="ps", bufs=2, space="PSUM"))
            ppd = ctx.enter_context(tc.tile_pool(name="pd", bufs=2, space="PSUM"))
            ppz = ctx.enter_context(tc.tile_pool(name="pz", bufs=2, space="PSUM"))
            ppb = ctx.enter_context(tc.tile_pool(name="pb", bufs=1, space="PSUM"))

            # persistent operands
            kA = pm.tile([128, L], F32R, tag="kA")
            nc.sync.dma_start(kA[:], kA_d)
            kB = pm.tile([16, L], F32R, tag="kB")
            nc.sync.dma_start(kB[:], kB_d)
            rnt = pm.tile([128, NCH], F32, tag="rnt")
            nc.sync.dma_start(rnt[:], rn_d)
            apt = {}
            for nm in range(9):
                for k in range(NCH):
                    t = pm.tile([128, 128], F32R, tag=f"ap{nm}_{k}")
                    nc.sync.dma_start(t[:], ap_d[nm * L + 128 * k: nm * L + 128 * (k + 1), :])
                    apt[(nm, k)] = t
            ones_f = pm.tile([1, 128], F32, tag="ones_f")
            nc.vector.memset(ones_f[:], 1.0)
            ones1 = pm.tile([1, 128], F32R, tag="ones1")   # lhsT for bcast [K=1,M=128]
            nc.vector.tensor_copy(ones1[:], ones_f[:])
            o128f = pm.tile([128, 1], F32, tag="o128f")
            nc.vector.memset(o128f[:], 1.0)
            o128 = pm.tile([128, 1], F32R, tag="o128")     # lhsT for Z [K=128,M=1]
            nc.vector.tensor_copy(o128[:], o128f[:])

            # att stripe buffers, persistent; pad cols 0,97 zeroed once
            att = []
            for k in range(NCH):
                t = pm.tile([128, RWS * 98], F32R, tag=f"att{k}")
                att.append(t)
                for pc in (0, 97):
                    nc.vector.memset(
                        t[:].rearrange("p (r c) -> p r c", c=98)[:, :, pc:pc + 1].bitcast(F32), 0.0)
                # stripe-0 boundary row (i=-1) zero
                nc.vector.memset(
                    t[:].rearrange("p (r c) -> p r c", c=98)[:, 0:1, :].bitcast(F32), 0.0)

            for st in range(NST):
                a0 = st * ST_A
                r_lo = 1 if st == 0 else 0
                r_hi = RWS - 1 if st == NST - 1 else RWS
                # q_col stripe load (valid i-rows only)
                i_lo = a0 - 1 + r_lo
                nrows = r_hi - r_lo
                qA = pq.tile([128, RWS * 96], F32R, tag="qA")
                qB = pq.tile([16, RWS * 96], F32R, tag="qB")
                nc.sync.dma_start(qA[:, r_lo * 96: (r_lo + nrows) * 96],
                                  qA_d[:, i_lo * 96: (i_lo + nrows) * 96])
                nc.sync.dma_start(qB[:, r_lo * 96: (r_lo + nrows) * 96],
                                  qB_d[:, i_lo * 96: (i_lo + nrows) * 96])
                if st == NST - 1:
                    for k in range(NCH):
                        nc.vector.memset(
                            att[k][:].rearrange("p (r c) -> p r c", c=98)
                            [:, RWS - 1:RWS, :].bitcast(F32), 0.0)

                # scores row-groups
                g1 = min(4, r_hi - r_lo)
                groups = [(r_lo, g1)]
                if r_hi - r_lo > g1:
                    groups.append((r_lo + g1, r_hi - r_lo - g1))
                for (rg, sz) in groups:
                    N = sz * 96
                    zps = ppz.tile([1, 512], F32, tag="zps")
                    for k in range(NCH):
                        ps = pps.tile([128, 512], F32, tag="ps")
                        nc.tensor.matmul(ps[:, :N], kA[:, 128 * k:128 * (k + 1)],
                                         qA[:, rg * 96: rg * 96 + N],
                                         start=True, stop=False)
                        nc.tensor.matmul(ps[:, :N], kB[:, 128 * k:128 * (k + 1)],
                                         qB[:, rg * 96: rg * 96 + N],
                                         start=False, stop=True)
                        # exp(10/norm_l * s) from psum -> att rows
                        dst = att[k][:].rearrange("p (r c) -> p r c", c=98)[:, rg:rg + sz, 1:97]
                        nc.scalar.activation(dst, ps[:, :N].rearrange("p (r c) -> p r c", c=96),
                                             AF.Exp, scale=rnt[:, k:k + 1])
                        # Z accumulation
                        nc.tensor.matmul(zps[:, :N], o128[:],
                                         att[k][:].rearrange("p (r c) -> p r c", c=98)
                                         [:, rg:rg + sz, 1:97],
                                         start=(k == 0), stop=(k == NCH - 1))
                    rz = prz.tile([1, 512], F32R, tag="rz")
                    with nc.allow_low_precision(reason="1/Z in f32r feeds matmul"):
                        nc.vector.reciprocal(rz[:, :N], zps[:, :N])
                    bps = ppb.tile([128, 512], F32, tag="bps")
                    nc.tensor.matmul(bps[:, :N], ones1[:], rz[:, :N], start=True, stop=True)
                    for k in range(NCH):
                        a_ap = att[k][:].rearrange("p (r c) -> p r c", c=98)[:, rg:rg + sz, 1:97]
                        nc.vector.tensor_mul(a_ap, a_ap,
                                             bps[:, :N].rearrange("p (r c) -> p r c", c=96))

                # deconv: 2 groups of 3 a-rows
                for g0 in (0, 3):
                    dps = ppd.tile([128, 288], F32, tag="dps")
                    first = True
                    for n in range(3):
                        for m in range(3):
                            nm = n * 3 + m
                            r0 = g0 + 2 - n
                            for k in range(NCH):
                                rhs = att[k][:].rearrange("p (r c) -> p r c", c=98)[:, r0:r0 + 3, 2 - m:98 - m]
                                nc.tensor.matmul(dps[:], apt[(nm, k)][:], rhs,
                                                 start=first,
                                                 stop=(nm == 8 and k == NCH - 1))
                                first = False
                    ob = pob.tile([128, 288], F32, tag="ob")
                    nc.vector.tensor_copy(ob[:], dps[:])
                    arow = a0 + g0
                    oap = oh_d.rearrange("p (y x) -> p y x", x=192)
                    nc.sync.dma_start(oap[:, arow:arow + 3, 0:192:2],
                                      ob[0:64, :].rearrange("p (r c) -> p r c", c=96))
                    nc.sync.dma_start(oap[:, arow:arow + 3, 1:192:2],
                                      ob[64:128, :].rearrange("p (r c) -> p r c", c=96))
    nc.compile()
    return nc


def _prelu(z, a):
    return np.where(z >= 0, z, a * z)


def _host_prep(x, wa, ba, aa, w1, b1, a1, w2, b2, a2):
    """Per-sample gather prep (numpy, validated vs reference)."""
    f32 = np.float32
    per_core = []
    waT_aug = (np.concatenate([wa.T, ba[None, :]], 0) / 6.0).astype(f32)
    w1T_aug = np.concatenate([w1.T, b1[None, :]], 0).astype(f32)
    w2T_aug = np.concatenate([w2.T / 4.0, b2[None, :]], 0).astype(f32)
    aav, a1v, a2v = float(aa[0]), float(a1[0]), float(a2[0])
    for s in range(B):
        xs = np.asarray(x[s], f32)
        xq_aug = np.concatenate([xs.reshape(64, -1), np.ones((1, H * W), f32)], 0)
        asmT = _prelu(xq_aug.T @ waT_aug, aav)
        qT = _prelu(xq_aug.T @ w1T_aug, a1v)
        x3 = xs.reshape(64, 96, 96)
        t1 = x3[:, :, 0::2] + x3[:, :, 1::2]
        xd = t1[:, 0::2, :] + t1[:, 1::2, :]
        xd_aug = np.concatenate([xd.reshape(64, -1), np.ones((1, 48 * 48), f32)], 0)
        kfT = _prelu(xd_aug.T @ w2T_aug, a2v)

        kf = kfT.T.reshape(Cr, 48, 48)
        kpT = np.zeros((144, L), f32)
        for t, (dy, dx) in enumerate([(a, b) for a in range(3) for b in range(3)]):
            ly_lo, ly_hi = max(0, 1 - dy), min(48, 49 - dy)
            lx_lo, lx_hi = max(0, 1 - dx), min(48, 49 - dx)
            blk = kf[:, ly_lo + dy - 1:ly_hi + dy - 1, lx_lo + dx - 1:lx_hi + dx - 1]
            dst = kpT[16 * t:16 * t + 16].reshape(Cr, 48, 48)
            dst[:, ly_lo:ly_hi, lx_lo:lx_hi] = blk
        nrm = np.sqrt((kpT ** 2).sum(0))
        rnorm10 = (10.0 / np.maximum(nrm, 1e-4)).astype(f32)
        rnt = rnorm10.reshape(NCH, 128).T.copy()           # [128, 18]

        q3 = qT.T.reshape(Cr, 96, 96)
        q_col = np.zeros((144, 96, 96), f32)
        for t, (dy, dx) in enumerate([(a, b) for a in range(3) for b in range(3)]):
            y_lo, y_hi = max(0, 1 - dy), min(96, 97 - dy)
            x_lo, x_hi = max(0, 1 - dx), min(96, 97 - dx)
            q_col[16 * t:16 * t + 16, y_lo:y_hi, x_lo:x_hi] = \
                q3[:, y_lo + dy - 1:y_hi + dy - 1, x_lo + dx - 1:x_hi + dx - 1]
        q_col = q_col.reshape(144, H * W)

        asm3 = asmT.T.reshape(64, 96, 96)
        for py in (0, 1):
            ap_t = np.zeros((3, 3, L, 128), f32)
            for n in range(3):
                u = py + 2 * n
                for m in range(3):
                    for half, v in ((0, 2 * m), (1, 2 * m + 1)):
                        ly_lo = max(0, (3 - u) // 2)
                        ly_hi = min(48, (99 - u) // 2)
                        lx_lo = max(0, (3 - v) // 2)
                        lx_hi = min(48, (97 - v) // 2 + 1)
                        Y0, X0 = 2 * ly_lo + u - 2, 2 * lx_lo + v - 2
                        blk = asm3[:, Y0:Y0 + 2 * (ly_hi - ly_lo):2,
                                   X0:X0 + 2 * (lx_hi - lx_lo):2]
                        dst = ap_t[n, m, :, 64 * half:64 * half + 64].reshape(48, 48, 64)
                        dst[ly_lo:ly_hi, lx_lo:lx_hi, :] = blk.transpose(1, 2, 0)
            per_core.append({
                "qcolA": np.ascontiguousarray(q_col[:128]),
                "qcolB": np.ascontiguousarray(q_col[128:144]),
                "kpTA": np.ascontiguousarray(kpT[:128]),
                "kpTB": np.ascontiguousarray(kpT[128:144]),
                "rnt": rnt,
                "ap": ap_t.reshape(9 * L, 128),
            })
    return per_core


def kernel(x, wa, ba, aa, w1, b1, a1, w2, b2, a2):
    global last_exec_time_ns
    if "nc" not in _cache:
        _cache["nc"] = _build_program()
    nc = _cache["nc"]
    in_maps = _host_prep(np.asarray(x, np.float32), np.asarray(wa), np.asarray(ba),
                         np.asarray(aa), np.asarray(w1), np.asarray(b1),
                         np.asarray(a1), np.asarray(w2), np.asarray(b2),
                         np.asarray(a2))
    import os
    trace = bool(int(os.environ.get("KERNEL_TRACE", "0")))
    res = run_bass_kernel_spmd(nc, in_maps, core_ids=list(range(8)), trace=trace)
    last_exec_time_ns = res.exec_time_ns
    out = np.zeros((B, C, 192, 192), np.float32)
    for core in range(8):
        s, py = core // 2, core % 2
        out[s, :, py::2, :] = res.results[core]["oh"].reshape(64, 96, 192)
    return out



# revision 7
# speedup vs baseline: 6.2924x; 6.2924x over previous
"""CrossScaleAttention Trainium2 kernel: 8-core SPMD via bass/tile.

Sharding: core (s, py) = (core//2, core%2): sample s = core//2, output row
parity py. Each core computes full attention for its sample and the deconv
for its output-row parity. Host prepares small gather tensors (q_col, kpT,
rnorm, ap taps — <0.1% of FLOPs); device does scores matmuls, softmax
(exp/Z/normalize) and the conv-transpose matmuls in fp32r.
"""
import sys, types
sys.path.insert(0, "/opt/trn_rl_repo")
import numpy as np
from contextlib import ExitStack

# NTFF profile hook shim (image's antenv lacks axon_hooks)
try:
    import trn_agent_boot.trn_boot as _tb
    _hook = _tb._ntff_profile_via_ctypes('/opt/axon/libaxon_pjrt.so')
    _m = types.ModuleType("antenv.axon_hooks")
    _m.get_axon_ntff_profile_hook = lambda: _hook
    _m.set_axon_ntff_profile_hook = lambda h: None
    sys.modules["antenv.axon_hooks"] = _m
except Exception:
    pass

import concourse.bass as bass
import concourse.tile as tile
import concourse.mybir as mybir
from concourse import bacc
from concourse.bass_utils import run_bass_kernel_spmd

F32 = mybir.dt.float32
F32R = mybir.dt.float32r
AF = mybir.ActivationFunctionType

C, Cr, B, H, W, L = 64, 16, 4, 96, 96, 2304
NCH = 18           # l-chunks of 128
ST_A = 6           # a-rows (output row-pairs) per stripe
RWS = ST_A + 2     # att i-rows buffered per stripe
NST = 96 // ST_A   # stripes

last_exec_time_ns = None

_cache = {}


def _build_program():
    nc = bacc.Bacc("TRN2", target_bir_lowering=False, debug=False, num_devices=8)
    qA_d = nc.dram_tensor("qcolA", [128, H * W], F32R, kind="ExternalInput").ap()
    qB_d = nc.dram_tensor("qcolB", [16, H * W], F32R, kind="ExternalInput").ap()
    kA_d = nc.dram_tensor("kpTA", [128, L], F32R, kind="ExternalInput").ap()
    kB_d = nc.dram_tensor("kpTB", [16, L], F32R, kind="ExternalInput").ap()
    rn_d = nc.dram_tensor("rnt", [128, NCH], F32, kind="ExternalInput").ap()
    ap_d = nc.dram_tensor("ap", [128, NCH * 9 * 128], F32R, kind="ExternalInput").ap()
    oh_d = nc.dram_tensor("oh", [64, 96 * 192], F32, kind="ExternalOutput").ap()

    with tile.TileContext(nc) as tc:
        with ExitStack() as ctx:
            pm = ctx.enter_context(tc.tile_pool(name="main", bufs=1))
            pq = ctx.enter_context(tc.tile_pool(name="q", bufs=2))
            pob = ctx.enter_context(tc.tile_pool(name="ob", bufs=3))
            prz = ctx.enter_context(tc.tile_pool(name="rz", bufs=2))
            pps = ctx.enter_context(tc.tile_pool(name="ps", bufs=2, space="PSUM"))
            ppd = ctx.enter_context(tc.tile_pool(name="pd", bufs=2, space="PSUM"))
            ppz = ctx.enter_context(tc.tile_pool(name="pz", bufs=2, space="PSUM"))
            ppb = ctx.enter_context(tc.tile_pool(name="pb", bufs=1, space="PSUM"))

            # persistent operands
            kA = pm.tile([128, L], F32R, tag="kA")
            nc.sync.dma_start(kA[:], kA_d)
            kB = pm.tile([16, L], F32R, tag="kB")
            nc.sync.dma_start(kB[:], kB_d)
            rnt = pm.tile([128, NCH], F32, tag="rnt")
            nc.sync.dma_start(rnt[:], rn_d)
            apall = pm.tile([128, NCH * 9 * 128], F32R, tag="apall")
            nc.sync.dma_start(apall[:], ap_d)
            apt = {(nm, k): apall[:, (k * 9 + nm) * 128: (k * 9 + nm) * 128 + 128]
                   for nm in range(9) for k in range(NCH)}
            ones_f = pm.tile([1, 128], F32, tag="ones_f")
            nc.vector.memset(ones_f[:], 1.0)
            ones1 = pm.tile([1, 128], F32R, tag="ones1")   # lhsT for bcast [K=1,M=128]
            nc.vector.tensor_copy(ones1[:], ones_f[:])
            o128f = pm.tile([128, 1], F32, tag="o128f")
            nc.vector.memset(o128f[:], 1.0)
            o128 = pm.tile([128, 1], F32R, tag="o128")     # lhsT for Z [K=128,M=1]
            nc.vector.tensor_copy(o128[:], o128f[:])

            # att stripe buffers, persistent; pad cols 0,97 zeroed once
            att = []
            for k in range(NCH):
                t = pm.tile([128, RWS * 98], F32R, tag=f"att{k}")
                att.append(t)
                for pc in (0, 97):
                    nc.vector.memset(
                        t[:].rearrange("p (r c) -> p r c", c=98)[:, :, pc:pc + 1].bitcast(F32), 0.0)
                # stripe-0 boundary row (i=-1) zero
                nc.vector.memset(
                    t[:].rearrange("p (r c) -> p r c", c=98)[:, 0:1, :].bitcast(F32), 0.0)

            for st in range(NST):
                a0 = st * ST_A
                r_lo = 1 if st == 0 else 0
                r_hi = RWS - 1 if st == NST - 1 else RWS
                # q_col stripe load (valid i-rows only)
                i_lo = a0 - 1 + r_lo
                nrows = r_hi - r_lo
                qA = pq.tile([128, RWS * 96], F32R, tag="qA")
                qB = pq.tile([16, RWS * 96], F32R, tag="qB")
                nc.sync.dma_start(qA[:, r_lo * 96: (r_lo + nrows) * 96],
                                  qA_d[:, i_lo * 96: (i_lo + nrows) * 96])
                nc.sync.dma_start(qB[:, r_lo * 96: (r_lo + nrows) * 96],
                                  qB_d[:, i_lo * 96: (i_lo + nrows) * 96])
                if st == NST - 1:
                    for k in range(NCH):
                        nc.vector.memset(
                            att[k][:].rearrange("p (r c) -> p r c", c=98)
                            [:, RWS - 1:RWS, :].bitcast(F32), 0.0)

                # scores row-groups
                g1 = min(4, r_hi - r_lo)
                groups = [(r_lo, g1)]
                if r_hi - r_lo > g1:
                    groups.append((r_lo + g1, r_hi - r_lo - g1))
                for (rg, sz) in groups:
                    N = sz * 96
                    zps = ppz.tile([1, 512], F32, tag="zps")
                    for k in range(NCH):
                        ps = pps.tile([128, 512], F32, tag="ps")
                        nc.tensor.matmul(ps[:, :N], kA[:, 128 * k:128 * (k + 1)],
                                         qA[:, rg * 96: rg * 96 + N],
                                         start=True, stop=False)
                        nc.tensor.matmul(ps[:, :N], kB[:, 128 * k:128 * (k + 1)],
                                         qB[:, rg * 96: rg * 96 + N],
                                         start=False, stop=True)
                        # exp(10/norm_l * s) from psum -> att rows
                        dst = att[k][:].rearrange("p (r c) -> p r c", c=98)[:, rg:rg + sz, 1:97]
                        nc.scalar.activation(dst, ps[:, :N].rearrange("p (r c) -> p r c", c=96),
                                             AF.Exp, scale=rnt[:, k:k + 1])
                        # Z accumulation
                        nc.tensor.matmul(zps[:, :N], o128[:],
                                         att[k][:].rearrange("p (r c) -> p r c", c=98)
                                         [:, rg:rg + sz, 1:97],
                                         start=(k == 0), stop=(k == NCH - 1))
                    rz = prz.tile([1, 512], F32R, tag="rz")
                    with nc.allow_low_precision(reason="1/Z in f32r feeds matmul"):
                        nc.vector.reciprocal(rz[:, :N], zps[:, :N])
                    bps = ppb.tile([128, 512], F32, tag="bps")
                    nc.tensor.matmul(bps[:, :N], ones1[:], rz[:, :N], start=True, stop=True)
                    for k in range(NCH):
                        a_ap = att[k][:].rearrange("p (r c) -> p r c", c=98)[:, rg:rg + sz, 1:97]
                        nc.vector.tensor_mul(a_ap, a_ap,
                                             bps[:, :N].rearrange("p (r c) -> p r c", c=96))

                # deconv: 2 groups of 3 a-rows
                for g0 in (0, 3):
                    dps = ppd.tile([128, 288], F32, tag="dps")
                    first = True
                    for n in range(3):
                        for m in range(3):
                            nm = n * 3 + m
                            r0 = g0 + 2 - n
                            for k in range(NCH):
                                rhs = att[k][:].rearrange("p (r c) -> p r c", c=98)[:, r0:r0 + 3, 2 - m:98 - m]
                                nc.tensor.matmul(dps[:], apt[(nm, k)], rhs,
                                                 start=first,
                                                 stop=(nm == 8 and k == NCH - 1))
                                first = False
                    ob = pob.tile([128, 288], F32, tag="ob")
                    nc.vector.tensor_copy(ob[:], dps[:])
                    arow = a0 + g0
                    # layout (y, xpar, x): contiguous 384B runs; host interleaves x
                    oap = oh_d.rearrange("p (y x) -> p y x", x=192)
                    nc.sync.dma_start(oap[:, arow:arow + 3, 0:96],
                                      ob[0:64, :].rearrange("p (r c) -> p r c", c=96))
                    nc.sync.dma_start(oap[:, arow:arow + 3, 96:192],
                                      ob[64:128, :].rearrange("p (r c) -> p r c", c=96))
    nc.compile()
    return nc


def _prelu(z, a):
    return np.where(z >= 0, z, a * z)


def _host_prep(x, wa, ba, aa, w1, b1, a1, w2, b2, a2):
    """Per-sample gather prep (numpy, validated vs reference)."""
    f32 = np.float32
    per_core = []
    waT_aug = (np.concatenate([wa.T, ba[None, :]], 0) / 6.0).astype(f32)
    w1T_aug = np.concatenate([w1.T, b1[None, :]], 0).astype(f32)
    w2T_aug = np.concatenate([w2.T / 4.0, b2[None, :]], 0).astype(f32)
    aav, a1v, a2v = float(aa[0]), float(a1[0]), float(a2[0])
    for s in range(B):
        xs = np.asarray(x[s], f32)
        xq_aug = np.concatenate([xs.reshape(64, -1), np.ones((1, H * W), f32)], 0)
        asmT = _prelu(xq_aug.T @ waT_aug, aav)
        qT = _prelu(xq_aug.T @ w1T_aug, a1v)
        x3 = xs.reshape(64, 96, 96)
        t1 = x3[:, :, 0::2] + x3[:, :, 1::2]
        xd = t1[:, 0::2, :] + t1[:, 1::2, :]
        xd_aug = np.concatenate([xd.reshape(64, -1), np.ones((1, 48 * 48), f32)], 0)
        kfT = _prelu(xd_aug.T @ w2T_aug, a2v)

        kf = kfT.T.reshape(Cr, 48, 48)
        kpT = np.zeros((144, L), f32)
        for t, (dy, dx) in enumerate([(a, b) for a in range(3) for b in range(3)]):
            ly_lo, ly_hi = max(0, 1 - dy), min(48, 49 - dy)
            lx_lo, lx_hi = max(0, 1 - dx), min(48, 49 - dx)
            blk = kf[:, ly_lo + dy - 1:ly_hi + dy - 1, lx_lo + dx - 1:lx_hi + dx - 1]
            dst = kpT[16 * t:16 * t + 16].reshape(Cr, 48, 48)
            dst[:, ly_lo:ly_hi, lx_lo:lx_hi] = blk
        nrm = np.sqrt((kpT ** 2).sum(0))
        rnorm10 = (10.0 / np.maximum(nrm, 1e-4)).astype(f32)
        rnt = rnorm10.reshape(NCH, 128).T.copy()           # [128, 18]

        q3 = qT.T.reshape(Cr, 96, 96)
        q_col = np.zeros((144, 96, 96), f32)
        for t, (dy, dx) in enumerate([(a, b) for a in range(3) for b in range(3)]):
            y_lo, y_hi = max(0, 1 - dy), min(96, 97 - dy)
            x_lo, x_hi = max(0, 1 - dx), min(96, 97 - dx)
            q_col[16 * t:16 * t + 16, y_lo:y_hi, x_lo:x_hi] = \
                q3[:, y_lo + dy - 1:y_hi + dy - 1, x_lo + dx - 1:x_hi + dx - 1]
        q_col = q_col.reshape(144, H * W)

        asm3 = asmT.T.reshape(64, 96, 96)
        for py in (0, 1):
            ap_t = np.zeros((3, 3, L, 128), f32)
            for n in range(3):
                u = py + 2 * n
                for m in range(3):
                    for half, v in ((0, 2 * m), (1, 2 * m + 1)):
                        ly_lo = max(0, (3 - u) // 2)
                        ly_hi = min(48, (99 - u) // 2)
                        lx_lo = max(0, (3 - v) // 2)
                        lx_hi = min(48, (97 - v) // 2 + 1)
                        Y0, X0 = 2 * ly_lo + u - 2, 2 * lx_lo + v - 2
                        blk = asm3[:, Y0:Y0 + 2 * (ly_hi - ly_lo):2,
                                   X0:X0 + 2 * (lx_hi - lx_lo):2]
                        dst = ap_t[n, m, :, 64 * half:64 * half + 64].reshape(48, 48, 64)
                        dst[ly_lo:ly_hi, lx_lo:lx_hi, :] = blk.transpose(1, 2, 0)
            # device ap layout: [p(128), k(18), nm(9), c(128)] for one-DMA load
            ap2 = np.ascontiguousarray(
                ap_t.reshape(9, NCH, 128, 128).transpose(2, 1, 0, 3)
            ).reshape(128, NCH * 9 * 128)
            per_core.append({
                "qcolA": np.ascontiguousarray(q_col[:128]),
                "qcolB": np.ascontiguousarray(q_col[128:144]),
                "kpTA": np.ascontiguousarray(kpT[:128]),
                "kpTB": np.ascontiguousarray(kpT[128:144]),
                "rnt": rnt,
                "ap": ap2,
            })
    return per_core


def kernel(x, wa, ba, aa, w1, b1, a1, w2, b2, a2):
    global last_exec_time_ns
    if "nc" not in _cache:
        _cache["nc"] = _build_program()
    nc = _cache["nc"]
    in_maps = _host_prep(np.asarray(x, np.float32), np.asarray(wa), np.asarray(ba),
                         np.asarray(aa), np.asarray(w1), np.asarray(b1),
                         np.asarray(a1), np.asarray(w2), np.asarray(b2),
                         np.asarray(a2))
    import os
    trace = bool(int(os.environ.get("KERNEL_TRACE", "0")))
    res = run_bass_kernel_spmd(nc, in_maps, core_ids=list(range(8)), trace=trace)
    last_exec_time_ns = res.exec_time_ns
    out = np.zeros((B, C, 192, 192), np.float32)
    for core in range(8):
        s, py = core // 2, core % 2
        r = res.results[core]["oh"].reshape(64, 96, 2, 96)
        out[s, :, py::2, 0::2] = r[:, :, 0, :]
        out[s, :, py::2, 1::2] = r[:, :, 1, :]
    return out



# revision 12
# speedup vs baseline: 7.5363x; 1.1977x over previous
"""CrossScaleAttention Trainium2 kernel: 8-core SPMD via bass/tile.

Sharding: core (s, py) = (core//2, core%2): sample s = core//2, output row
parity py. Each core computes full attention for its sample and the deconv
for its output-row parity. Host prepares small gather tensors (q_col, kpT,
ap taps — <0.1% of FLOPs); device does scores matmuls (fp16), softmax
(exp/Z/normalize, bf16 att) and the conv-transpose matmuls (bf16).

Schedule: double-buffered att stripes; the deconv matmuls of stripe s-1 are
interleaved into the ACT-bound softmax phase of stripe s so the PE never
idles waiting on exp.
"""
import sys, types
sys.path.insert(0, "/opt/trn_rl_repo")
import numpy as np
import ml_dtypes
from contextlib import ExitStack

# NTFF profile hook shim (image's antenv lacks axon_hooks)
try:
    import trn_agent_boot.trn_boot as _tb
    _hook = _tb._ntff_profile_via_ctypes('/opt/axon/libaxon_pjrt.so')
    _m = types.ModuleType("antenv.axon_hooks")
    _m.get_axon_ntff_profile_hook = lambda: _hook
    _m.set_axon_ntff_profile_hook = lambda h: None
    sys.modules["antenv.axon_hooks"] = _m
except Exception:
    pass

import concourse.bass as bass
import concourse.tile as tile
import concourse.mybir as mybir
from concourse import bacc
from concourse.bass_utils import run_bass_kernel_spmd

F32 = mybir.dt.float32
F32R = mybir.dt.float32r
F16 = mybir.dt.float16
BF16 = mybir.dt.bfloat16
AF = mybir.ActivationFunctionType

C, Cr, B, H, W, L = 64, 16, 4, 96, 96, 2304
NCH = 18           # l-chunks of 128
ST_A = 6           # a-rows (output row-pairs) per stripe
RWS = ST_A + 2     # att i-rows buffered per stripe
NST = 96 // ST_A   # stripes

last_exec_time_ns = None

_cache = {}


def _build_program():
    nc = bacc.Bacc("TRN2", target_bir_lowering=False, debug=False, num_devices=8)
    qA_d = nc.dram_tensor("qcolA", [128, H * W], F16, kind="ExternalInput").ap()
    qB_d = nc.dram_tensor("qcolB", [16, H * W], F16, kind="ExternalInput").ap()
    kA_d = nc.dram_tensor("kpTA", [128, L], F16, kind="ExternalInput").ap()
    kB_d = nc.dram_tensor("kpTB", [16, L], F16, kind="ExternalInput").ap()
    ap_d = nc.dram_tensor("ap", [128, NCH * 9 * 128], BF16, kind="ExternalInput").ap()
    oh_d = nc.dram_tensor("oh", [64, 96 * 192], F32, kind="ExternalOutput").ap()

    with tile.TileContext(nc) as tc:
        with ExitStack() as ctx:
            pm = ctx.enter_context(tc.tile_pool(name="main", bufs=1))
            pq = ctx.enter_context(tc.tile_pool(name="q", bufs=2))
            pob = ctx.enter_context(tc.tile_pool(name="ob", bufs=3))
            prz = ctx.enter_context(tc.tile_pool(name="rz", bufs=2))
            pps = ctx.enter_context(tc.tile_pool(name="ps", bufs=4, space="PSUM"))
            ppd = ctx.enter_context(tc.tile_pool(name="pd", bufs=2, space="PSUM"))
            ppz = ctx.enter_context(tc.tile_pool(name="pz", bufs=1, space="PSUM"))
            ppb = ctx.enter_context(tc.tile_pool(name="pb", bufs=1, space="PSUM"))

            # persistent operands
            kA = pm.tile([128, L], F16, tag="kA")
            nc.sync.dma_start(kA[:], kA_d)
            kB = pm.tile([16, L], F16, tag="kB")
            nc.sync.dma_start(kB[:], kB_d)
            apall = pm.tile([128, NCH * 9 * 128], BF16, tag="apall")
            nc.sync.dma_start(apall[:], ap_d)
            ones_f = pm.tile([1, 128], F32, tag="ones_f")
            nc.vector.memset(ones_f[:], 1.0)
            ones1 = pm.tile([1, 128], F32R, tag="ones1")   # lhsT for bcast [K=1,M=128]
            nc.vector.tensor_copy(ones1[:], ones_f[:])
            o128f = pm.tile([128, 1], F32, tag="o128f")
            nc.vector.memset(o128f[:], 1.0)
            o128 = pm.tile([128, 1], BF16, tag="o128")     # lhsT for Z [K=128,M=1]
            nc.vector.tensor_copy(o128[:], o128f[:])

            # att stripe buffers (double-buffered), bf16, one big tile each:
            # layout per partition: [k(NCH), r(RWS), c(98)]; cols 0,97 are pad
            attb = []
            for h in range(2):
                t = pm.tile([128, NCH * RWS * 98], BF16, tag=f"att{h}")
                attb.append(t)

            def chunk_view(h, k):
                return attb[h][:, k * RWS * 98:(k + 1) * RWS * 98] \
                    .rearrange("p (r c) -> p r c", c=98)

            for h in range(2):
                for k in range(NCH):
                    v = chunk_view(h, k)
                    for pc in (0, 97):
                        nc.vector.memset(v[:, :, pc:pc + 1], 0.0)
                    # stripe-0 boundary row (i=-1) zero
                    nc.vector.memset(v[:, 0:1, :], 0.0)

            # deconv MM emitters: one stripe-parity group = 162 accumulating MMs
            def deconv_mms(h, g0):
                """Operand list for the 162 matmuls of a 3-a-row deconv group."""
                out = []
                for n in range(3):
                    for m in range(3):
                        nm = n * 3 + m
                        r0 = g0 + 2 - n
                        for k in range(NCH):
                            off = (k * 9 + nm) * 128
                            rhs = chunk_view(h, k)[:, r0:r0 + 3, 2 - m:98 - m]
                            out.append((apall[:, off:off + 128], rhs))
                return out

            # state of the pending (previous-stripe) deconv
            pending = None   # (h, arow, mm list, next index, dps tile)

            def emit_deconv_slice(cnt):
                nonlocal pending
                if pending is None:
                    return
                h, arow, mms, idx, dps = pending
                end = min(idx + cnt, len(mms))
                for i in range(idx, end):
                    lw, rhs = mms[i]
                    nc.tensor.matmul(dps[:], lw, rhs,
                                     start=(i == 0), stop=(i == len(mms) - 1))
                if end == len(mms):
                    ob = pob.tile([128, 288], F32, tag="ob")
                    nc.vector.tensor_copy(ob[:], dps[:])
                    oap = oh_d.rearrange("p (y x) -> p y x", x=192)
                    nc.sync.dma_start(oap[:, arow:arow + 3, 0:96],
                                      ob[0:64, :].rearrange("p (r c) -> p r c", c=96))
                    nc.sync.dma_start(oap[:, arow:arow + 3, 96:192],
                                      ob[64:128, :].rearrange("p (r c) -> p r c", c=96))
                    pending = None
                else:
                    pending = (h, arow, mms, end, dps)

            def start_deconv(h, arow, g0):
                nonlocal pending
                assert pending is None
                dps = ppd.tile([128, 288], F32, tag="dps")
                pending = (h, arow, deconv_mms(h, g0), 0, dps)

            deconv_queue = []   # (h, arow, g0) groups not yet started

            for st in range(NST):
                h = st % 2
                a0 = st * ST_A
                r_lo = 1 if st == 0 else 0
                r_hi = RWS - 1 if st == NST - 1 else RWS
                i_lo = a0 - 1 + r_lo
                nrows = r_hi - r_lo
                qA = pq.tile([128, RWS * 96], F16, tag="qA")
                qB = pq.tile([16, RWS * 96], F16, tag="qB")
                nc.sync.dma_start(qA[:, r_lo * 96: (r_lo + nrows) * 96],
                                  qA_d[:, i_lo * 96: (i_lo + nrows) * 96])
                nc.sync.dma_start(qB[:, r_lo * 96: (r_lo + nrows) * 96],
                                  qB_d[:, i_lo * 96: (i_lo + nrows) * 96])
                if st == NST - 1:
                    # boundary row (i=96) zero, this buffer's last row
                    for k in range(NCH):
                        nc.vector.memset(chunk_view(h, k)[:, RWS - 1:RWS, :], 0.0)

                g1 = min(4, r_hi - r_lo)
                groups = [(r_lo, g1)]
                if r_hi - r_lo > g1:
                    groups.append((r_lo + g1, r_hi - r_lo - g1))
                for (rg, sz) in groups:
                    N = sz * 96
                    # start one pending deconv group from the backlog
                    if pending is None and deconv_queue:
                        start_deconv(*deconv_queue.pop(0))
                    zps = ppz.tile([1, 512], F32, tag="zps")
                    for k in range(NCH):
                        ps = pps.tile([128, 512], F32, tag="ps")
                        nc.tensor.matmul(ps[:, :N], kA[:, 128 * k:128 * (k + 1)],
                                         qA[:, rg * 96: rg * 96 + N],
                                         start=True, stop=False)
                        nc.tensor.matmul(ps[:, :N], kB[:, 128 * k:128 * (k + 1)],
                                         qB[:, rg * 96: rg * 96 + N],
                                         start=False, stop=True)
                        # fill PE with previous-stripe deconv while ACT exps
                        emit_deconv_slice(9)
                        # exp(s) from psum -> att rows (scale folded into kpT)
                        dst = chunk_view(h, k)[:, rg:rg + sz, 1:97]
                        nc.scalar.activation(dst, ps[:, :N].rearrange("p (r c) -> p r c", c=96),
                                             AF.Exp)
                        # Z accumulation (exp_k long done by the time PE gets here)
                        nc.tensor.matmul(zps[:, :N], o128[:], dst,
                                         start=(k == 0), stop=(k == NCH - 1))
                    rz = prz.tile([1, 512], F32R, tag="rz")
                    with nc.allow_low_precision(reason="1/Z in f32r feeds matmul"):
                        nc.vector.reciprocal(rz[:, :N], zps[:, :N])
                    bps = ppb.tile([128, 512], F32, tag="bps")
                    nc.tensor.matmul(bps[:, :N], ones1[:], rz[:, :N], start=True, stop=True)
                    for k in range(NCH):
                        a_ap = chunk_view(h, k)[:, rg:rg + sz, 1:97]
                        nc.vector.tensor_mul(a_ap, a_ap,
                                             bps[:, :N].rearrange("p (r c) -> p r c", c=96))

                # queue this stripe's deconv groups (run during next stripe)
                for g0 in (0, 3):
                    deconv_queue.append((h, a0 + g0, g0))
                # drain any unfinished pending deconv before stripe ends?
                # no — let it continue into the next stripe's blocks.

            # drain remaining deconv groups
            while pending is not None or deconv_queue:
                if pending is None:
                    start_deconv(*deconv_queue.pop(0))
                emit_deconv_slice(10 ** 9)
    nc.compile()
    return nc


def _prelu(z, a):
    return np.where(z >= 0, z, a * z)


def _host_prep(x, wa, ba, aa, w1, b1, a1, w2, b2, a2):
    """Per-sample gather prep (numpy, validated vs reference)."""
    f32 = np.float32
    per_core = []
    waT_aug = (np.concatenate([wa.T, ba[None, :]], 0) / 6.0).astype(f32)
    w1T_aug = np.concatenate([w1.T, b1[None, :]], 0).astype(f32)
    w2T_aug = np.concatenate([w2.T / 4.0, b2[None, :]], 0).astype(f32)
    aav, a1v, a2v = float(aa[0]), float(a1[0]), float(a2[0])
    for s in range(B):
        xs = np.asarray(x[s], f32)
        xq_aug = np.concatenate([xs.reshape(64, -1), np.ones((1, H * W), f32)], 0)
        asmT = _prelu(xq_aug.T @ waT_aug, aav)
        qT = _prelu(xq_aug.T @ w1T_aug, a1v)
        x3 = xs.reshape(64, 96, 96)
        t1 = x3[:, :, 0::2] + x3[:, :, 1::2]
        xd = t1[:, 0::2, :] + t1[:, 1::2, :]
        xd_aug = np.concatenate([xd.reshape(64, -1), np.ones((1, 48 * 48), f32)], 0)
        kfT = _prelu(xd_aug.T @ w2T_aug, a2v)

        kf = kfT.T.reshape(Cr, 48, 48)
        kpT = np.zeros((144, L), f32)
        for t, (dy, dx) in enumerate([(a, b) for a in range(3) for b in range(3)]):
            ly_lo, ly_hi = max(0, 1 - dy), min(48, 49 - dy)
            lx_lo, lx_hi = max(0, 1 - dx), min(48, 49 - dx)
            blk = kf[:, ly_lo + dy - 1:ly_hi + dy - 1, lx_lo + dx - 1:lx_hi + dx - 1]
            dst = kpT[16 * t:16 * t + 16].reshape(Cr, 48, 48)
            dst[:, ly_lo:ly_hi, lx_lo:lx_hi] = blk
        nrm = np.sqrt((kpT ** 2).sum(0))
        rnorm10 = (10.0 / np.maximum(nrm, 1e-4)).astype(f32)
        # fold the softmax scale / norm into kpT: scores psum = 10*s/norm
        kpT = kpT * rnorm10[None, :]

        q3 = qT.T.reshape(Cr, 96, 96)
        q_col = np.zeros((144, 96, 96), f32)
        for t, (dy, dx) in enumerate([(a, b) for a in range(3) for b in range(3)]):
            y_lo, y_hi = max(0, 1 - dy), min(96, 97 - dy)
            x_lo, x_hi = max(0, 1 - dx), min(96, 97 - dx)
            q_col[16 * t:16 * t + 16, y_lo:y_hi, x_lo:x_hi] = \
                q3[:, y_lo + dy - 1:y_hi + dy - 1, x_lo + dx - 1:x_hi + dx - 1]
        q_col = q_col.reshape(144, H * W)

        asm3 = asmT.T.reshape(64, 96, 96)
        for py in (0, 1):
            ap_t = np.zeros((3, 3, L, 128), f32)
            for n in range(3):
                u = py + 2 * n
                for m in range(3):
                    for half, v in ((0, 2 * m), (1, 2 * m + 1)):
                        ly_lo = max(0, (3 - u) // 2)
                        ly_hi = min(48, (99 - u) // 2)
                        lx_lo = max(0, (3 - v) // 2)
                        lx_hi = min(48, (97 - v) // 2 + 1)
                        Y0, X0 = 2 * ly_lo + u - 2, 2 * lx_lo + v - 2
                        blk = asm3[:, Y0:Y0 + 2 * (ly_hi - ly_lo):2,
                                   X0:X0 + 2 * (lx_hi - lx_lo):2]
                        dst = ap_t[n, m, :, 64 * half:64 * half + 64].reshape(48, 48, 64)
                        dst[ly_lo:ly_hi, lx_lo:lx_hi, :] = blk.transpose(1, 2, 0)
            # device ap layout: [p(128), k(18), nm(9), c(128)] for one-DMA load
            ap2 = np.ascontiguousarray(
                ap_t.reshape(9, NCH, 128, 128).transpose(2, 1, 0, 3)
            ).reshape(128, NCH * 9 * 128).astype(ml_dtypes.bfloat16)
            per_core.append({
                "qcolA": np.ascontiguousarray(q_col[:128]).astype(np.float16),
                "qcolB": np.ascontiguousarray(q_col[128:144]).astype(np.float16),
                "kpTA": np.ascontiguousarray(kpT[:128]).astype(np.float16),
                "kpTB": np.ascontiguousarray(kpT[128:144]).astype(np.float16),
                "ap": ap2,
            })
    return per_core


def kernel(x, wa, ba, aa, w1, b1, a1, w2, b2, a2):
    global last_exec_time_ns
    if "nc" not in _cache:
        _cache["nc"] = _build_program()
    nc = _cache["nc"]
    in_maps = _host_prep(np.asarray(x, np.float32), np.asarray(wa), np.asarray(ba),
                         np.asarray(aa), np.asarray(w1), np.asarray(b1),
                         np.asarray(a1), np.asarray(w2), np.asarray(b2),
                         np.asarray(a2))
    import os
    trace = bool(int(os.environ.get("KERNEL_TRACE", "0")))
    res = run_bass_kernel_spmd(nc, in_maps, core_ids=list(range(8)), trace=trace)
    last_exec_time_ns = res.exec_time_ns
    out = np.zeros((B, C, 192, 192), np.float32)
    for core in range(8):
        s, py = core // 2, core % 2
        r = res.results[core]["oh"].reshape(64, 96, 2, 96)
        out[s, :, py::2, 0::2] = r[:, :, 0, :]
        out[s, :, py::2, 1::2] = r[:, :, 1, :]
    return out


# revision 19
# speedup vs baseline: 7.8497x; 1.0416x over previous
"""CrossScaleAttention Trainium2 kernel: 8-core SPMD via bass/tile.

Sharding: core (s, py) = (core//2, core%2): sample s = core//2, output row
parity py. Each core computes full attention for its sample and the deconv
for its output-row parity. Host prepares small gather tensors (q_col, kpT,
ap taps — <0.1% of FLOPs); device does scores matmuls (fp16), softmax
(exp/Z/normalize, bf16 att) and the conv-transpose matmuls (bf16).

Schedule: double-buffered att stripes; the deconv matmuls of stripe s-1 are
interleaved into the ACT-bound softmax phase of stripe s so the PE never
idles waiting on exp.
"""
import sys, types
sys.path.insert(0, "/opt/trn_rl_repo")
import numpy as np
import ml_dtypes
from contextlib import ExitStack

# NTFF profile hook shim (image's antenv lacks axon_hooks)
try:
    import trn_agent_boot.trn_boot as _tb
    _hook = _tb._ntff_profile_via_ctypes('/opt/axon/libaxon_pjrt.so')
    _m = types.ModuleType("antenv.axon_hooks")
    _m.get_axon_ntff_profile_hook = lambda: _hook
    _m.set_axon_ntff_profile_hook = lambda h: None
    sys.modules["antenv.axon_hooks"] = _m
except Exception:
    pass

import concourse.bass as bass
import concourse.tile as tile
import concourse.mybir as mybir
from concourse import bacc
from concourse.bass_utils import run_bass_kernel_spmd

F32 = mybir.dt.float32
F32R = mybir.dt.float32r
F16 = mybir.dt.float16
BF16 = mybir.dt.bfloat16
AF = mybir.ActivationFunctionType

C, Cr, B, H, W, L = 64, 16, 4, 96, 96, 2304
NCH = 18           # l-chunks of 128
ST_A = 12          # a-rows (output row-pairs) per stripe
RWS = ST_A + 2     # att i-rows buffered per stripe
NST = 96 // ST_A   # stripes

last_exec_time_ns = None

_cache = {}


def _build_program():
    nc = bacc.Bacc("TRN2", target_bir_lowering=False, debug=False, num_devices=8)
    qA_d = nc.dram_tensor("qcolA", [128, H * W], F16, kind="ExternalInput").ap()
    qB_d = nc.dram_tensor("qcolB", [16, H * W], F16, kind="ExternalInput").ap()
    kA_d = nc.dram_tensor("kpTA", [128, L], F16, kind="ExternalInput").ap()
    kB_d = nc.dram_tensor("kpTB", [16, L], F16, kind="ExternalInput").ap()
    ap_d = nc.dram_tensor("ap", [128, NCH * 9 * 128], BF16, kind="ExternalInput").ap()
    oh_d = nc.dram_tensor("oh", [64, 96 * 192], F32, kind="ExternalOutput").ap()

    with tile.TileContext(nc) as tc:
        with ExitStack() as ctx:
            pm = ctx.enter_context(tc.tile_pool(name="main", bufs=1))
            pq = ctx.enter_context(tc.tile_pool(name="q", bufs=2))
            pob = ctx.enter_context(tc.tile_pool(name="ob", bufs=3))
            prz = ctx.enter_context(tc.tile_pool(name="rz", bufs=2))
            pps = ctx.enter_context(tc.tile_pool(name="ps", bufs=4, space="PSUM"))
            ppd = ctx.enter_context(tc.tile_pool(name="pd", bufs=2, space="PSUM"))
            ppz = ctx.enter_context(tc.tile_pool(name="pz", bufs=1, space="PSUM"))
            ppb = ctx.enter_context(tc.tile_pool(name="pb", bufs=1, space="PSUM"))

            # persistent operands
            kA = pm.tile([128, L], F16, tag="kA")
            nc.sync.dma_start(kA[:], kA_d)
            kB = pm.tile([16, L], F16, tag="kB")
            nc.sync.dma_start(kB[:], kB_d)
            apall = pm.tile([128, NCH * 9 * 128], BF16, tag="apall")
            nc.sync.dma_start(apall[:], ap_d)
            ones_f = pm.tile([1, 128], F32, tag="ones_f")
            nc.vector.memset(ones_f[:], 1.0)
            ones1 = pm.tile([1, 128], F32R, tag="ones1")   # lhsT for bcast [K=1,M=128]
            nc.vector.tensor_copy(ones1[:], ones_f[:])
            o128f = pm.tile([128, 1], F32, tag="o128f")
            nc.vector.memset(o128f[:], 1.0)
            o128 = pm.tile([128, 1], BF16, tag="o128")     # lhsT for Z [K=128,M=1]
            nc.vector.tensor_copy(o128[:], o128f[:])

            # att stripe buffers (double-buffered), bf16, one big tile each:
            # layout per partition: [k(NCH), r(RWS), c(98)]; cols 0,97 are pad
            attb = []
            for h in range(2):
                t = pm.tile([128, NCH * RWS * 98], BF16, tag=f"att{h}")
                attb.append(t)

            def chunk_view(h, k):
                return attb[h][:, k * RWS * 98:(k + 1) * RWS * 98] \
                    .rearrange("p (r c) -> p r c", c=98)

            for h in range(2):
                for k in range(NCH):
                    v = chunk_view(h, k)
                    for pc in (0, 97):
                        nc.vector.memset(v[:, :, pc:pc + 1], 0.0)
                    # stripe-0 boundary row (i=-1) zero
                    nc.vector.memset(v[:, 0:1, :], 0.0)

            # deconv MM emitters: one stripe-parity group = 162 accumulating MMs
            # (k outer so the normalize->deconv dependency ramps one chunk at
            # a time instead of needing 9 chunks normalized up front)
            def deconv_mms(h, g0):
                """Operand list for the 162 matmuls of a 3-a-row deconv group."""
                out = []
                for k in range(NCH):
                    v = chunk_view(h, k)
                    for n in range(3):
                        for m in range(3):
                            nm = n * 3 + m
                            r0 = g0 + 2 - n
                            off = (k * 9 + nm) * 128
                            rhs = v[:, r0:r0 + 3, 2 - m:98 - m]
                            out.append((apall[:, off:off + 128], rhs))
                return out

            # state of the pending (previous-stripe) deconv
            pending = None   # (h, arow, mm list, next index, dps tile)

            def emit_deconv_slice(cnt):
                nonlocal pending
                while cnt > 0:
                    if pending is None:
                        if not deconv_queue:
                            return
                        start_deconv(*deconv_queue.pop(0))
                    h, arow, mms, idx, dps = pending
                    end = min(idx + cnt, len(mms))
                    for i in range(idx, end):
                        lw, rhs = mms[i]
                        nc.tensor.matmul(dps[:], lw, rhs,
                                         start=(i == 0), stop=(i == len(mms) - 1))
                    cnt -= end - idx
                    if end == len(mms):
                        ob = pob.tile([128, 288], F32, tag="ob")
                        nc.scalar.activation(ob[:], dps[:], AF.Copy)
                        oap = oh_d.rearrange("p (y x) -> p y x", x=192)
                        nc.sync.dma_start(oap[:, arow:arow + 3, 0:96],
                                          ob[0:64, :].rearrange("p (r c) -> p r c", c=96))
                        nc.sync.dma_start(oap[:, arow:arow + 3, 96:192],
                                          ob[64:128, :].rearrange("p (r c) -> p r c", c=96))
                        pending = None
                    else:
                        pending = (h, arow, mms, end, dps)

            def start_deconv(h, arow, g0):
                nonlocal pending
                assert pending is None
                dps = ppd.tile([128, 288], F32, tag="dps")
                pending = (h, arow, deconv_mms(h, g0), 0, dps)

            deconv_queue = []   # (h, arow, g0) groups not yet started

            for st in range(NST):
                h = st % 2
                a0 = st * ST_A
                r_lo = 1 if st == 0 else 0
                r_hi = RWS - 1 if st == NST - 1 else RWS
                i_lo = a0 - 1 + r_lo
                nrows = r_hi - r_lo
                qA = pq.tile([128, RWS * 96], F16, tag="qA")
                qB = pq.tile([16, RWS * 96], F16, tag="qB")
                nc.sync.dma_start(qA[:, r_lo * 96: (r_lo + nrows) * 96],
                                  qA_d[:, i_lo * 96: (i_lo + nrows) * 96])
                nc.sync.dma_start(qB[:, r_lo * 96: (r_lo + nrows) * 96],
                                  qB_d[:, i_lo * 96: (i_lo + nrows) * 96])
                if st == NST - 1:
                    # boundary row (i=96) zero, this buffer's last row
                    for k in range(NCH):
                        nc.vector.memset(chunk_view(h, k)[:, RWS - 1:RWS, :], 0.0)

                groups = []
                r = r_lo
                while r < r_hi:
                    sz = min(5, r_hi - r)
                    groups.append((r, sz))
                    r += sz
                for (rg, sz) in groups:
                    N = sz * 96
                    zps = ppz.tile([1, 512], F32, tag="zps")
                    for k in range(NCH):
                        ps = pps.tile([128, 512], F32, tag="ps")
                        nc.tensor.matmul(ps[:, :N], kA[:, 128 * k:128 * (k + 1)],
                                         qA[:, rg * 96: rg * 96 + N],
                                         start=True, stop=False)
                        nc.tensor.matmul(ps[:, :N], kB[:, 128 * k:128 * (k + 1)],
                                         qB[:, rg * 96: rg * 96 + N],
                                         start=False, stop=True)
                        # fill PE with previous-stripe deconv while ACT exps
                        if pending is None and deconv_queue:
                            start_deconv(*deconv_queue.pop(0))
                        emit_deconv_slice(12)
                        # exp(s) from psum -> att rows (scale folded into kpT)
                        dst = chunk_view(h, k)[:, rg:rg + sz, 1:97]
                        nc.scalar.activation(dst, ps[:, :N].rearrange("p (r c) -> p r c", c=96),
                                             AF.Exp)
                        # Z accumulation (exp_k long done by the time PE gets here)
                        nc.tensor.matmul(zps[:, :N], o128[:], dst,
                                         start=(k == 0), stop=(k == NCH - 1))
                    rz = prz.tile([1, 512], F32R, tag="rz")
                    with nc.allow_low_precision(reason="1/Z in f32r feeds matmul"):
                        nc.vector.reciprocal(rz[:, :N], zps[:, :N])
                    bps = ppb.tile([128, 512], F32, tag="bps")
                    nc.tensor.matmul(bps[:, :N], ones1[:], rz[:, :N], start=True, stop=True)
                    # stage 1/Z to SBUF bf16 so the muls run in DVE 2x mode
                    bsb = prz.tile([128, 512], BF16, tag="bsb")
                    nc.scalar.activation(bsb[:, :N], bps[:, :N], AF.Copy)
                    for k in range(NCH):
                        a_ap = chunk_view(h, k)[:, rg:rg + sz, 1:97]
                        nc.vector.tensor_mul(a_ap, a_ap,
                                             bsb[:, :N].rearrange("p (r c) -> p r c", c=96))

                # queue this stripe's deconv groups (run during next stripe)
                for g0 in range(0, ST_A, 3):
                    deconv_queue.append((h, a0 + g0, g0))
                # drain any unfinished pending deconv before stripe ends?
                # no — let it continue into the next stripe's blocks.

            # drain remaining deconv groups
            while pending is not None or deconv_queue:
                if pending is None:
                    start_deconv(*deconv_queue.pop(0))
                emit_deconv_slice(10 ** 9)
    nc.compile()
    return nc


def _prelu(z, a):
    return np.where(z >= 0, z, a * z)


def _host_prep(x, wa, ba, aa, w1, b1, a1, w2, b2, a2):
    """Per-sample gather prep (numpy, validated vs reference)."""
    f32 = np.float32
    per_core = []
    waT_aug = (np.concatenate([wa.T, ba[None, :]], 0) / 6.0).astype(f32)
    w1T_aug = np.concatenate([w1.T, b1[None, :]], 0).astype(f32)
    w2T_aug = np.concatenate([w2.T / 4.0, b2[None, :]], 0).astype(f32)
    aav, a1v, a2v = float(aa[0]), float(a1[0]), float(a2[0])
    for s in range(B):
        xs = np.asarray(x[s], f32)
        xq_aug = np.concatenate([xs.reshape(64, -1), np.ones((1, H * W), f32)], 0)
        asmT = _prelu(xq_aug.T @ waT_aug, aav)
        qT = _prelu(xq_aug.T @ w1T_aug, a1v)
        x3 = xs.reshape(64, 96, 96)
        t1 = x3[:, :, 0::2] + x3[:, :, 1::2]
        xd = t1[:, 0::2, :] + t1[:, 1::2, :]
        xd_aug = np.concatenate([xd.reshape(64, -1), np.ones((1, 48 * 48), f32)], 0)
        kfT = _prelu(xd_aug.T @ w2T_aug, a2v)

        kf = kfT.T.reshape(Cr, 48, 48)
        kpT = np.zeros((144, L), f32)
        for t, (dy, dx) in enumerate([(a, b) for a in range(3) for b in range(3)]):
            ly_lo, ly_hi = max(0, 1 - dy), min(48, 49 - dy)
            lx_lo, lx_hi = max(0, 1 - dx), min(48, 49 - dx)
            blk = kf[:, ly_lo + dy - 1:ly_hi + dy - 1, lx_lo + dx - 1:lx_hi + dx - 1]
            dst = kpT[16 * t:16 * t + 16].reshape(Cr, 48, 48)
            dst[:, ly_lo:ly_hi, lx_lo:lx_hi] = blk
        nrm = np.sqrt((kpT ** 2).sum(0))
        rnorm10 = (10.0 / np.maximum(nrm, 1e-4)).astype(f32)
        # fold the softmax scale / norm into kpT: scores psum = 10*s/norm
        kpT = kpT * rnorm10[None, :]

        q3 = qT.T.reshape(Cr, 96, 96)
        q_col = np.zeros((144, 96, 96), f32)
        for t, (dy, dx) in enumerate([(a, b) for a in range(3) for b in range(3)]):
            y_lo, y_hi = max(0, 1 - dy), min(96, 97 - dy)
            x_lo, x_hi = max(0, 1 - dx), min(96, 97 - dx)
            q_col[16 * t:16 * t + 16, y_lo:y_hi, x_lo:x_hi] = \
                q3[:, y_lo + dy - 1:y_hi + dy - 1, x_lo + dx - 1:x_hi + dx - 1]
        q_col = q_col.reshape(144, H * W)

        asm3 = asmT.T.reshape(64, 96, 96)
        for py in (0, 1):
            ap_t = np.zeros((3, 3, L, 128), f32)
            for n in range(3):
                u = py + 2 * n
                for m in range(3):
                    for half, v in ((0, 2 * m), (1, 2 * m + 1)):
                        ly_lo = max(0, (3 - u) // 2)
                        ly_hi = min(48, (99 - u) // 2)
                        lx_lo = max(0, (3 - v) // 2)
                        lx_hi = min(48, (97 - v) // 2 + 1)
                        Y0, X0 = 2 * ly_lo + u - 2, 2 * lx_lo + v - 2
                        blk = asm3[:, Y0:Y0 + 2 * (ly_hi - ly_lo):2,
                                   X0:X0 + 2 * (lx_hi - lx_lo):2]
                        dst = ap_t[n, m, :, 64 * half:64 * half + 64].reshape(48, 48, 64)
                        dst[ly_lo:ly_hi, lx_lo:lx_hi, :] = blk.transpose(1, 2, 0)
            # device ap layout: [p(128), k(18), nm(9), c(128)] for one-DMA load
            ap2 = np.ascontiguousarray(
                ap_t.reshape(9, NCH, 128, 128).transpose(2, 1, 0, 3)
            ).reshape(128, NCH * 9 * 128).astype(ml_dtypes.bfloat16)
            per_core.append({
                "qcolA": np.ascontiguousarray(q_col[:128]).astype(np.float16),
                "qcolB": np.ascontiguousarray(q_col[128:144]).astype(np.float16),
                "kpTA": np.ascontiguousarray(kpT[:128]).astype(np.float16),
                "kpTB": np.ascontiguousarray(kpT[128:144]).astype(np.float16),
                "ap": ap2,
            })
    return per_core


def kernel(x, wa, ba, aa, w1, b1, a1, w2, b2, a2):
    global last_exec_time_ns
    if "nc" not in _cache:
        _cache["nc"] = _build_program()
    nc = _cache["nc"]
    in_maps = _host_prep(np.asarray(x, np.float32), np.asarray(wa), np.asarray(ba),
                         np.asarray(aa), np.asarray(w1), np.asarray(b1),
                         np.asarray(a1), np.asarray(w2), np.asarray(b2),
                         np.asarray(a2))
    import os
    trace = bool(int(os.environ.get("KERNEL_TRACE", "0")))
    res = run_bass_kernel_spmd(nc, in_maps, core_ids=list(range(8)), trace=trace)
    last_exec_time_ns = res.exec_time_ns
    out = np.zeros((B, C, 192, 192), np.float32)
    for core in range(8):
        s, py = core // 2, core % 2
        r = res.results[core]["oh"].reshape(64, 96, 2, 96)
        out[s, :, py::2, 0::2] = r[:, :, 0, :]
        out[s, :, py::2, 1::2] = r[:, :, 1, :]
    return out


# revision 25
# speedup vs baseline: 8.7797x; 1.1185x over previous
"""CrossScaleAttention Trainium2 kernel: 8-core SPMD via bass/tile.

Sharding: core (s, py) = (core//2, core%2): sample s = core//2, output row
parity py. Each core computes full attention for its sample and the deconv
for its output-row parity. Host prepares small gather tensors (q_col, kpT,
ap taps — <0.1% of FLOPs); device does scores matmuls (fp16), softmax
(exp/Z/normalize, bf16 att) and the conv-transpose matmuls (bf16).

Schedule: double-buffered att stripes; the deconv matmuls of stripe s-1 are
interleaved into the ACT-bound softmax phase of stripe s so the PE never
idles waiting on exp.
"""
import sys, types
sys.path.insert(0, "/opt/trn_rl_repo")
import numpy as np
import ml_dtypes
from contextlib import ExitStack

# NTFF profile hook shim (image's antenv lacks axon_hooks)
try:
    import trn_agent_boot.trn_boot as _tb
    _hook = _tb._ntff_profile_via_ctypes('/opt/axon/libaxon_pjrt.so')
    _m = types.ModuleType("antenv.axon_hooks")
    _m.get_axon_ntff_profile_hook = lambda: _hook
    _m.set_axon_ntff_profile_hook = lambda h: None
    sys.modules["antenv.axon_hooks"] = _m
except Exception:
    pass

import concourse.bass as bass
import concourse.tile as tile
import concourse.mybir as mybir
from concourse import bacc
from concourse.bass_utils import run_bass_kernel_spmd

F32 = mybir.dt.float32
F32R = mybir.dt.float32r
F16 = mybir.dt.float16
BF16 = mybir.dt.bfloat16
AF = mybir.ActivationFunctionType

C, Cr, B, H, W, L = 64, 16, 4, 96, 96, 2304
NCH = 18           # l-chunks of 128
ST_A = 12          # a-rows (output row-pairs) per stripe
RWS = ST_A + 2     # att i-rows buffered per stripe
NST = 96 // ST_A   # stripes

last_exec_time_ns = None

_cache = {}


def _build_program():
    nc = bacc.Bacc("TRN2", target_bir_lowering=False, debug=False, num_devices=8)
    qA_d = nc.dram_tensor("qcolA", [128, H * W], F16, kind="ExternalInput").ap()
    qB_d = nc.dram_tensor("qcolB", [16, H * W], F16, kind="ExternalInput").ap()
    kA_d = nc.dram_tensor("kpTA", [128, L], F16, kind="ExternalInput").ap()
    kB_d = nc.dram_tensor("kpTB", [16, L], F16, kind="ExternalInput").ap()
    ap_d = nc.dram_tensor("ap", [128, NCH * 9 * 128], BF16, kind="ExternalInput").ap()
    oh_d = nc.dram_tensor("oh", [64, 96 * 192], F32, kind="ExternalOutput").ap()

    with tile.TileContext(nc) as tc:
        with ExitStack() as ctx:
            pm = ctx.enter_context(tc.tile_pool(name="main", bufs=1))
            pq = ctx.enter_context(tc.tile_pool(name="q", bufs=2))
            pob = ctx.enter_context(tc.tile_pool(name="ob", bufs=3))
            prz = ctx.enter_context(tc.tile_pool(name="rz", bufs=2))
            pps = ctx.enter_context(tc.tile_pool(name="ps", bufs=3, space="PSUM"))
            ppd = ctx.enter_context(tc.tile_pool(name="pd", bufs=2, space="PSUM"))
            ppz = ctx.enter_context(tc.tile_pool(name="pz", bufs=2, space="PSUM"))
            ppb = ctx.enter_context(tc.tile_pool(name="pb", bufs=1, space="PSUM"))

            # persistent operands
            kA = pm.tile([128, L], F16, tag="kA")
            nc.sync.dma_start(kA[:], kA_d)
            kB = pm.tile([16, L], F16, tag="kB")
            nc.sync.dma_start(kB[:], kB_d)
            apall = pm.tile([128, NCH * 9 * 128], BF16, tag="apall")
            nc.sync.dma_start(apall[:], ap_d)
            ones_f = pm.tile([1, 128], F32, tag="ones_f")
            nc.vector.memset(ones_f[:], 1.0)
            ones1 = pm.tile([1, 128], F32R, tag="ones1")   # lhsT for bcast [K=1,M=128]
            nc.vector.tensor_copy(ones1[:], ones_f[:])
            o128f = pm.tile([128, 1], F32, tag="o128f")
            nc.vector.memset(o128f[:], 1.0)
            o128 = pm.tile([128, 1], BF16, tag="o128")     # lhsT for Z [K=128,M=1]
            nc.vector.tensor_copy(o128[:], o128f[:])

            # att stripe buffers (double-buffered), bf16, one big tile each:
            # layout per partition: [k(NCH), r(RWS), c(98)]; cols 0,97 are pad
            attb = []
            for h in range(2):
                t = pm.tile([128, NCH * RWS * 98], BF16, tag=f"att{h}")
                attb.append(t)

            def chunk_view(h, k):
                return attb[h][:, k * RWS * 98:(k + 1) * RWS * 98] \
                    .rearrange("p (r c) -> p r c", c=98)

            for h in range(2):
                for k in range(NCH):
                    v = chunk_view(h, k)
                    for pc in (0, 97):
                        nc.vector.memset(v[:, :, pc:pc + 1], 0.0)
                    # stripe-0 boundary row (i=-1) zero
                    nc.vector.memset(v[:, 0:1, :], 0.0)

            # deconv MM emitters: one group = 162 accumulating MMs over G a-rows
            # (k outer so the normalize->deconv dependency ramps one chunk at
            # a time instead of needing 9 chunks normalized up front)
            def deconv_mms(h, g0, G):
                """Operand list for the 162 matmuls of a G-a-row deconv group."""
                out = []
                for k in range(NCH):
                    v = chunk_view(h, k)
                    for n in range(3):
                        for m in range(3):
                            nm = n * 3 + m
                            r0 = g0 + 2 - n
                            off = (k * 9 + nm) * 128
                            rhs = v[:, r0:r0 + G, 2 - m:98 - m]
                            out.append((apall[:, off:off + 128], rhs))
                return out

            # state of the pending (previous-stripe) deconv
            pending = None   # (h, arow, G, mm list, next index, dps tile)
            deferred = None  # deferred normalize tail of the previous group

            def emit_deconv_slice(cnt):
                nonlocal pending
                while cnt > 0:
                    if pending is None:
                        if not deconv_queue:
                            return
                        start_deconv(*deconv_queue.pop(0))
                    h, arow, G, mms, idx, dps = pending
                    end = min(idx + cnt, len(mms))
                    for i in range(idx, end):
                        lw, rhs = mms[i]
                        nc.tensor.matmul(dps[:, :G * 96], lw, rhs,
                                         start=(i == 0), stop=(i == len(mms) - 1))
                    cnt -= end - idx
                    if end == len(mms):
                        ob = pob.tile([128, 480], F32, tag="ob")
                        nc.scalar.activation(ob[:, :G * 96], dps[:, :G * 96], AF.Copy)
                        oap = oh_d.rearrange("p (y x) -> p y x", x=192)
                        nc.sync.dma_start(oap[:, arow:arow + G, 0:96],
                                          ob[0:64, :G * 96].rearrange("p (r c) -> p r c", c=96))
                        nc.sync.dma_start(oap[:, arow:arow + G, 96:192],
                                          ob[64:128, :G * 96].rearrange("p (r c) -> p r c", c=96))
                        pending = None
                    else:
                        pending = (h, arow, G, mms, end, dps)

            def start_deconv(h, arow, g0, G):
                nonlocal pending
                assert pending is None
                dps = ppd.tile([128, 480], F32, tag="dps")
                pending = (h, arow, G, deconv_mms(h, g0, G), 0, dps)

            deconv_queue = []   # (h, arow, g0, G) groups not yet started

            for st in range(NST):
                h = st % 2
                a0 = st * ST_A
                r_lo = 1 if st == 0 else 0
                r_hi = RWS - 1 if st == NST - 1 else RWS
                i_lo = a0 - 1 + r_lo
                nrows = r_hi - r_lo
                qA = pq.tile([128, RWS * 96], F16, tag="qA")
                qB = pq.tile([16, RWS * 96], F16, tag="qB")
                nc.sync.dma_start(qA[:, r_lo * 96: (r_lo + nrows) * 96],
                                  qA_d[:, i_lo * 96: (i_lo + nrows) * 96])
                nc.sync.dma_start(qB[:, r_lo * 96: (r_lo + nrows) * 96],
                                  qB_d[:, i_lo * 96: (i_lo + nrows) * 96])
                if st == NST - 1:
                    # boundary row (i=96) zero, this buffer's last row
                    for k in range(NCH):
                        nc.vector.memset(chunk_view(h, k)[:, RWS - 1:RWS, :], 0.0)

                groups = []
                r = r_lo
                while r < r_hi:
                    sz = min(5, r_hi - r)
                    groups.append((r, sz))
                    r += sz
                for (rg, sz) in groups:
                    N = sz * 96
                    zps = ppz.tile([1, 512], F32, tag="zps")
                    for k in range(NCH):
                        ps = pps.tile([128, 512], F32, tag="ps")
                        nc.tensor.matmul(ps[:, :N], kA[:, 128 * k:128 * (k + 1)],
                                         qA[:, rg * 96: rg * 96 + N],
                                         start=True, stop=False)
                        nc.tensor.matmul(ps[:, :N], kB[:, 128 * k:128 * (k + 1)],
                                         qB[:, rg * 96: rg * 96 + N],
                                         start=False, stop=True)
                        # fill PE with previous-stripe deconv while ACT exps
                        emit_deconv_slice(12)
                        # exp(s) from psum -> att rows (scale folded into kpT)
                        dst = chunk_view(h, k)[:, rg:rg + sz, 1:97]
                        nc.scalar.activation(dst, ps[:, :N].rearrange("p (r c) -> p r c", c=96),
                                             AF.Exp)
                        # Z accumulation (exp_k long done by the time PE gets here)
                        nc.tensor.matmul(zps[:, :N], o128[:], dst,
                                         start=(k == 0), stop=(k == NCH - 1))
                    rz = prz.tile([1, 512], F32R, tag="rz")
                    with nc.allow_low_precision(reason="1/Z in f32r feeds matmul"):
                        nc.vector.reciprocal(rz[:, :N], zps[:, :N])
                    bps = ppb.tile([128, 512], F32, tag="bps")
                    nc.tensor.matmul(bps[:, :N], ones1[:], rz[:, :N], start=True, stop=True)
                    # stage 1/Z to SBUF bf16 so the muls run in DVE 2x mode
                    bsb = prz.tile([128, 512], BF16, tag="bsb")
                    nc.scalar.activation(bsb[:, :N], bps[:, :N], AF.Copy)
                    for k in range(NCH):
                        a_ap = chunk_view(h, k)[:, rg:rg + sz, 1:97]
                        nc.vector.tensor_mul(a_ap, a_ap,
                                             bsb[:, :N].rearrange("p (r c) -> p r c", c=96))

                # queue this stripe's deconv groups (run during next stripe)
                for g0 in range(0, ST_A, 3):
                    deconv_queue.append((h, a0 + g0, g0))
                # drain any unfinished pending deconv before stripe ends?
                # no — let it continue into the next stripe's blocks.

            # drain remaining deconv groups
            emit_deconv_slice(10 ** 9)
    nc.compile()
    return nc


def _prelu(z, a):
    return np.where(z >= 0, z, a * z)


def _host_prep(x, wa, ba, aa, w1, b1, a1, w2, b2, a2):
    """Per-sample gather prep (numpy, validated vs reference)."""
    f32 = np.float32
    per_core = []
    waT_aug = (np.concatenate([wa.T, ba[None, :]], 0) / 6.0).astype(f32)
    w1T_aug = np.concatenate([w1.T, b1[None, :]], 0).astype(f32)
    w2T_aug = np.concatenate([w2.T / 4.0, b2[None, :]], 0).astype(f32)
    aav, a1v, a2v = float(aa[0]), float(a1[0]), float(a2[0])
    for s in range(B):
        xs = np.asarray(x[s], f32)
        xq_aug = np.concatenate([xs.reshape(64, -1), np.ones((1, H * W), f32)], 0)
        asmT = _prelu(xq_aug.T @ waT_aug, aav)
        qT = _prelu(xq_aug.T @ w1T_aug, a1v)
        x3 = xs.reshape(64, 96, 96)
        t1 = x3[:, :, 0::2] + x3[:, :, 1::2]
        xd = t1[:, 0::2, :] + t1[:, 1::2, :]
        xd_aug = np.concatenate([xd.reshape(64, -1), np.ones((1, 48 * 48), f32)], 0)
        kfT = _prelu(xd_aug.T @ w2T_aug, a2v)

        kf = kfT.T.reshape(Cr, 48, 48)
        kpT = np.zeros((144, L), f32)
        for t, (dy, dx) in enumerate([(a, b) for a in range(3) for b in range(3)]):
            ly_lo, ly_hi = max(0, 1 - dy), min(48, 49 - dy)
            lx_lo, lx_hi = max(0, 1 - dx), min(48, 49 - dx)
            blk = kf[:, ly_lo + dy - 1:ly_hi + dy - 1, lx_lo + dx - 1:lx_hi + dx - 1]
            dst = kpT[16 * t:16 * t + 16].reshape(Cr, 48, 48)
            dst[:, ly_lo:ly_hi, lx_lo:lx_hi] = blk
        nrm = np.sqrt((kpT ** 2).sum(0))
        rnorm10 = (10.0 / np.maximum(nrm, 1e-4)).astype(f32)
        # fold the softmax scale / norm into kpT: scores psum = 10*s/norm
        kpT = kpT * rnorm10[None, :]

        q3 = qT.T.reshape(Cr, 96, 96)
        q_col = np.zeros((144, 96, 96), f32)
        for t, (dy, dx) in enumerate([(a, b) for a in range(3) for b in range(3)]):
            y_lo, y_hi = max(0, 1 - dy), min(96, 97 - dy)
            x_lo, x_hi = max(0, 1 - dx), min(96, 97 - dx)
            q_col[16 * t:16 * t + 16, y_lo:y_hi, x_lo:x_hi] = \
                q3[:, y_lo + dy - 1:y_hi + dy - 1, x_lo + dx - 1:x_hi + dx - 1]
        q_col = q_col.reshape(144, H * W)

        asm3 = asmT.T.reshape(64, 96, 96)
        for py in (0, 1):
            ap_t = np.zeros((3, 3, L, 128), f32)
            for n in range(3):
                u = py + 2 * n
                for m in range(3):
                    for half, v in ((0, 2 * m), (1, 2 * m + 1)):
                        ly_lo = max(0, (3 - u) // 2)
                        ly_hi = min(48, (99 - u) // 2)
                        lx_lo = max(0, (3 - v) // 2)
                        lx_hi = min(48, (97 - v) // 2 + 1)
                        Y0, X0 = 2 * ly_lo + u - 2, 2 * lx_lo + v - 2
                        blk = asm3[:, Y0:Y0 + 2 * (ly_hi - ly_lo):2,
                                   X0:X0 + 2 * (lx_hi - lx_lo):2]
                        dst = ap_t[n, m, :, 64 * half:64 * half + 64].reshape(48, 48, 64)
                        dst[ly_lo:ly_hi, lx_lo:lx_hi, :] = blk.transpose(1, 2, 0)
            # device ap layout: [p(128), k(18), nm(9), c(128)] for one-DMA load
            ap2 = np.ascontiguousarray(
                ap_t.reshape(9, NCH, 128, 128).transpose(2, 1, 0, 3)
            ).reshape(128, NCH * 9 * 128).astype(ml_dtypes.bfloat16)
            per_core.append({
                "qcolA": np.ascontiguousarray(q_col[:128]).astype(np.float16),
                "qcolB": np.ascontiguousarray(q_col[128:144]).astype(np.float16),
                "kpTA": np.ascontiguousarray(kpT[:128]).astype(np.float16),
                "kpTB": np.ascontiguousarray(kpT[128:144]).astype(np.float16),
                "ap": ap2,
            })
    return per_core


def kernel(x, wa, ba, aa, w1, b1, a1, w2, b2, a2):
    global last_exec_time_ns
    if "nc" not in _cache:
        _cache["nc"] = _build_program()
    nc = _cache["nc"]
    in_maps = _host_prep(np.asarray(x, np.float32), np.asarray(wa), np.asarray(ba),
                         np.asarray(aa), np.asarray(w1), np.asarray(b1),
                         np.asarray(a1), np.asarray(w2), np.asarray(b2),
                         np.asarray(a2))
    import os
    trace = bool(int(os.environ.get("KERNEL_TRACE", "0")))
    res = run_bass_kernel_spmd(nc, in_maps, core_ids=list(range(8)), trace=trace)
    last_exec_time_ns = res.exec_time_ns
    out = np.zeros((B, C, 192, 192), np.float32)
    for core in range(8):
        s, py = core // 2, core % 2
        r = res.results[core]["oh"].reshape(64, 96, 2, 96)
        out[s, :, py::2, 0::2] = r[:, :, 0, :]
        out[s, :, py::2, 1::2] = r[:, :, 1, :]
    return out


# revision 27
# speedup vs baseline: 9.2235x; 1.0505x over previous
"""CrossScaleAttention Trainium2 kernel: 8-core SPMD via bass/tile.

Sharding: core (s, py) = (core//2, core%2): sample s = core//2, output row
parity py. Each core computes full attention for its sample and the deconv
for its output-row parity. Host prepares small gather tensors (q_col, kpT,
ap taps — <0.1% of FLOPs); device does scores matmuls (fp16), softmax
(exp/Z/normalize, bf16 att) and the conv-transpose matmuls (bf16).

Schedule: double-buffered att stripes; the deconv matmuls of stripe s-1 are
interleaved into the ACT-bound softmax phase of stripe s so the PE never
idles waiting on exp.
"""
import sys, types
sys.path.insert(0, "/opt/trn_rl_repo")
import numpy as np
import ml_dtypes
from contextlib import ExitStack

# NTFF profile hook shim (image's antenv lacks axon_hooks)
try:
    import trn_agent_boot.trn_boot as _tb
    _hook = _tb._ntff_profile_via_ctypes('/opt/axon/libaxon_pjrt.so')
    _m = types.ModuleType("antenv.axon_hooks")
    _m.get_axon_ntff_profile_hook = lambda: _hook
    _m.set_axon_ntff_profile_hook = lambda h: None
    sys.modules["antenv.axon_hooks"] = _m
except Exception:
    pass

import concourse.bass as bass
import concourse.tile as tile
import concourse.mybir as mybir
from concourse import bacc
from concourse.bass_utils import run_bass_kernel_spmd

F32 = mybir.dt.float32
F32R = mybir.dt.float32r
F16 = mybir.dt.float16
BF16 = mybir.dt.bfloat16
AF = mybir.ActivationFunctionType

C, Cr, B, H, W, L = 64, 16, 4, 96, 96, 2304
NCH = 18           # l-chunks of 128
ST_A = 12          # a-rows (output row-pairs) per stripe
RWS = ST_A + 2     # att i-rows buffered per stripe
NST = 96 // ST_A   # stripes

last_exec_time_ns = None

_cache = {}


def _build_program():
    nc = bacc.Bacc("TRN2", target_bir_lowering=False, debug=False, num_devices=8)
    qA_d = nc.dram_tensor("qcolA", [128, H * W], F16, kind="ExternalInput").ap()
    qB_d = nc.dram_tensor("qcolB", [16, H * W], F16, kind="ExternalInput").ap()
    kA_d = nc.dram_tensor("kpTA", [128, L], F16, kind="ExternalInput").ap()
    kB_d = nc.dram_tensor("kpTB", [16, L], F16, kind="ExternalInput").ap()
    ap_d = nc.dram_tensor("ap", [128, NCH * 9 * 128], BF16, kind="ExternalInput").ap()
    oh_d = nc.dram_tensor("oh", [64, 96 * 192], F32, kind="ExternalOutput").ap()

    with tile.TileContext(nc) as tc:
        with ExitStack() as ctx:
            pm = ctx.enter_context(tc.tile_pool(name="main", bufs=1))
            pq = ctx.enter_context(tc.tile_pool(name="q", bufs=2))
            pob = ctx.enter_context(tc.tile_pool(name="ob", bufs=3))
            prz = ctx.enter_context(tc.tile_pool(name="rz", bufs=2))
            pps = ctx.enter_context(tc.tile_pool(name="ps", bufs=3, space="PSUM"))
            ppd = ctx.enter_context(tc.tile_pool(name="pd", bufs=2, space="PSUM"))
            ppz = ctx.enter_context(tc.tile_pool(name="pz", bufs=2, space="PSUM"))
            ppb = ctx.enter_context(tc.tile_pool(name="pb", bufs=1, space="PSUM"))

            # persistent operands
            kA = pm.tile([128, L], F16, tag="kA")
            nc.sync.dma_start(kA[:], kA_d)
            kB = pm.tile([16, L], F16, tag="kB")
            nc.sync.dma_start(kB[:], kB_d)
            apall = pm.tile([128, NCH * 9 * 128], BF16, tag="apall")
            nc.sync.dma_start(apall[:], ap_d)
            ones_f = pm.tile([1, 128], F32, tag="ones_f")
            nc.vector.memset(ones_f[:], 1.0)
            ones1 = pm.tile([1, 128], F32R, tag="ones1")   # lhsT for bcast [K=1,M=128]
            nc.vector.tensor_copy(ones1[:], ones_f[:])
            o128f = pm.tile([128, 1], F32, tag="o128f")
            nc.vector.memset(o128f[:], 1.0)
            o128 = pm.tile([128, 1], BF16, tag="o128")     # lhsT for Z [K=128,M=1]
            nc.vector.tensor_copy(o128[:], o128f[:])

            # att stripe buffers (double-buffered), bf16, one big tile each:
            # layout per partition: [k(NCH), r(RWS), c(98)]; cols 0,97 are pad
            attb = []
            for h in range(2):
                t = pm.tile([128, NCH * RWS * 98], BF16, tag=f"att{h}")
                attb.append(t)

            def chunk_view(h, k):
                return attb[h][:, k * RWS * 98:(k + 1) * RWS * 98] \
                    .rearrange("p (r c) -> p r c", c=98)

            for h in range(2):
                for k in range(NCH):
                    v = chunk_view(h, k)
                    for pc in (0, 97):
                        nc.vector.memset(v[:, :, pc:pc + 1], 0.0)
                    # stripe-0 boundary row (i=-1) zero
                    nc.vector.memset(v[:, 0:1, :], 0.0)

            # deconv MM emitters: one group = 162 accumulating MMs over G a-rows
            # (k outer so the normalize->deconv dependency ramps one chunk at
            # a time instead of needing 9 chunks normalized up front)
            def deconv_mms(h, g0, G):
                """Operand list for the 162 matmuls of a G-a-row deconv group."""
                out = []
                for k in range(NCH):
                    v = chunk_view(h, k)
                    for n in range(3):
                        for m in range(3):
                            nm = n * 3 + m
                            r0 = g0 + 2 - n
                            off = (k * 9 + nm) * 128
                            rhs = v[:, r0:r0 + G, 2 - m:98 - m]
                            out.append((apall[:, off:off + 128], rhs))
                return out

            # state of the pending (previous-stripe) deconv
            pending = None   # (h, arow, G, mm list, next index, dps tile)
            deferred = None  # deferred normalize tail of the previous group

            def emit_deconv_slice(cnt):
                nonlocal pending
                while cnt > 0:
                    if pending is None:
                        if not deconv_queue:
                            return
                        start_deconv(*deconv_queue.pop(0))
                    h, arow, G, mms, idx, dps = pending
                    end = min(idx + cnt, len(mms))
                    for i in range(idx, end):
                        lw, rhs = mms[i]
                        nc.tensor.matmul(dps[:, :G * 96], lw, rhs,
                                         start=(i == 0), stop=(i == len(mms) - 1))
                    cnt -= end - idx
                    if end == len(mms):
                        ob = pob.tile([128, 480], F32, tag="ob")
                        nc.scalar.activation(ob[:, :G * 96], dps[:, :G * 96], AF.Copy)
                        oap = oh_d.rearrange("p (y x) -> p y x", x=192)
                        nc.sync.dma_start(oap[:, arow:arow + G, 0:96],
                                          ob[0:64, :G * 96].rearrange("p (r c) -> p r c", c=96))
                        nc.sync.dma_start(oap[:, arow:arow + G, 96:192],
                                          ob[64:128, :G * 96].rearrange("p (r c) -> p r c", c=96))
                        pending = None
                    else:
                        pending = (h, arow, G, mms, end, dps)

            def start_deconv(h, arow, g0, G):
                nonlocal pending
                assert pending is None
                dps = ppd.tile([128, 480], F32, tag="dps")
                pending = (h, arow, G, deconv_mms(h, g0, G), 0, dps)

            deconv_queue = []   # (h, arow, g0, G) groups not yet started

            for st in range(NST):
                h = st % 2
                a0 = st * ST_A
                r_lo = 1 if st == 0 else 0
                r_hi = RWS - 1 if st == NST - 1 else RWS
                i_lo = a0 - 1 + r_lo
                nrows = r_hi - r_lo
                qA = pq.tile([128, RWS * 96], F16, tag="qA")
                qB = pq.tile([16, RWS * 96], F16, tag="qB")
                nc.sync.dma_start(qA[:, r_lo * 96: (r_lo + nrows) * 96],
                                  qA_d[:, i_lo * 96: (i_lo + nrows) * 96])
                nc.sync.dma_start(qB[:, r_lo * 96: (r_lo + nrows) * 96],
                                  qB_d[:, i_lo * 96: (i_lo + nrows) * 96])
                if st == NST - 1:
                    # boundary row (i=96) zero, this buffer's last row
                    for k in range(NCH):
                        nc.vector.memset(chunk_view(h, k)[:, RWS - 1:RWS, :], 0.0)

                groups = []
                r = r_lo
                while r < r_hi:
                    sz = min(5, r_hi - r)
                    groups.append((r, sz))
                    r += sz
                for (rg, sz) in groups:
                    N = sz * 96
                    zps = ppz.tile([1, 512], F32, tag="zps")
                    for k in range(NCH):
                        ps = pps.tile([128, 512], F32, tag="ps")
                        nc.tensor.matmul(ps[:, :N], kA[:, 128 * k:128 * (k + 1)],
                                         qA[:, rg * 96: rg * 96 + N],
                                         start=True, stop=False)
                        nc.tensor.matmul(ps[:, :N], kB[:, 128 * k:128 * (k + 1)],
                                         qB[:, rg * 96: rg * 96 + N],
                                         start=False, stop=True)
                        # fill PE with previous-stripe deconv while ACT exps
                        emit_deconv_slice(9)
                        if k == 3 and deferred is not None:
                            deferred()
                            deferred = None
                        # exp(s) from psum -> att rows (scale folded into kpT)
                        dst = chunk_view(h, k)[:, rg:rg + sz, 1:97]
                        nc.scalar.activation(dst, ps[:, :N].rearrange("p (r c) -> p r c", c=96),
                                             AF.Exp)
                        # Z accumulation (exp_k long done by the time PE gets here)
                        nc.tensor.matmul(zps[:, :N], o128[:], dst,
                                         start=(k == 0), stop=(k == NCH - 1))
                    rz = prz.tile([1, 512], F32R, tag="rz")
                    with nc.allow_low_precision(reason="1/Z in f32r feeds matmul"):
                        nc.vector.reciprocal(rz[:, :N], zps[:, :N])

                    def tail(h=h, rg=rg, sz=sz, N=N, rz=rz):
                        # normalize: broadcast 1/Z, stage to bf16, scale att
                        bps = ppb.tile([128, 512], F32, tag="bps")
                        nc.tensor.matmul(bps[:, :N], ones1[:], rz[:, :N],
                                         start=True, stop=True)
                        bsb = prz.tile([128, 512], BF16, tag="bsb")
                        nc.scalar.activation(bsb[:, :N], bps[:, :N], AF.Copy)
                        for k in range(NCH):
                            a_ap = chunk_view(h, k)[:, rg:rg + sz, 1:97]
                            nc.vector.tensor_mul(a_ap, a_ap,
                                                 bsb[:, :N].rearrange("p (r c) -> p r c", c=96))
                    deferred = tail

                # queue this stripe's deconv groups (run during next stripe)
                g0 = 0
                while g0 < ST_A:
                    G = min(5, ST_A - g0)
                    deconv_queue.append((h, a0 + g0, g0, G))
                    g0 += G
                # drain any unfinished pending deconv before stripe ends?
                # no — let it continue into the next stripe's blocks.

            # flush the last normalize tail, then drain remaining deconv groups
            if deferred is not None:
                deferred()
                deferred = None
            emit_deconv_slice(10 ** 9)
    nc.compile()
    return nc


def _prelu(z, a):
    return np.where(z >= 0, z, a * z)


def _host_prep(x, wa, ba, aa, w1, b1, a1, w2, b2, a2):
    """Per-sample gather prep (numpy, validated vs reference)."""
    f32 = np.float32
    per_core = []
    waT_aug = (np.concatenate([wa.T, ba[None, :]], 0) / 6.0).astype(f32)
    w1T_aug = np.concatenate([w1.T, b1[None, :]], 0).astype(f32)
    w2T_aug = np.concatenate([w2.T / 4.0, b2[None, :]], 0).astype(f32)
    aav, a1v, a2v = float(aa[0]), float(a1[0]), float(a2[0])
    for s in range(B):
        xs = np.asarray(x[s], f32)
        xq_aug = np.concatenate([xs.reshape(64, -1), np.ones((1, H * W), f32)], 0)
        asmT = _prelu(xq_aug.T @ waT_aug, aav)
        qT = _prelu(xq_aug.T @ w1T_aug, a1v)
        x3 = xs.reshape(64, 96, 96)
        t1 = x3[:, :, 0::2] + x3[:, :, 1::2]
        xd = t1[:, 0::2, :] + t1[:, 1::2, :]
        xd_aug = np.concatenate([xd.reshape(64, -1), np.ones((1, 48 * 48), f32)], 0)
        kfT = _prelu(xd_aug.T @ w2T_aug, a2v)

        kf = kfT.T.reshape(Cr, 48, 48)
        kpT = np.zeros((144, L), f32)
        for t, (dy, dx) in enumerate([(a, b) for a in range(3) for b in range(3)]):
            ly_lo, ly_hi = max(0, 1 - dy), min(48, 49 - dy)
            lx_lo, lx_hi = max(0, 1 - dx), min(48, 49 - dx)
            blk = kf[:, ly_lo + dy - 1:ly_hi + dy - 1, lx_lo + dx - 1:lx_hi + dx - 1]
            dst = kpT[16 * t:16 * t + 16].reshape(Cr, 48, 48)
            dst[:, ly_lo:ly_hi, lx_lo:lx_hi] = blk
        nrm = np.sqrt((kpT ** 2).sum(0))
        rnorm10 = (10.0 / np.maximum(nrm, 1e-4)).astype(f32)
        # fold the softmax scale / norm into kpT: scores psum = 10*s/norm
        kpT = kpT * rnorm10[None, :]

        q3 = qT.T.reshape(Cr, 96, 96)
        q_col = np.zeros((144, 96, 96), f32)
        for t, (dy, dx) in enumerate([(a, b) for a in range(3) for b in range(3)]):
            y_lo, y_hi = max(0, 1 - dy), min(96, 97 - dy)
            x_lo, x_hi = max(0, 1 - dx), min(96, 97 - dx)
            q_col[16 * t:16 * t + 16, y_lo:y_hi, x_lo:x_hi] = \
                q3[:, y_lo + dy - 1:y_hi + dy - 1, x_lo + dx - 1:x_hi + dx - 1]
        q_col = q_col.reshape(144, H * W)

        asm3 = asmT.T.reshape(64, 96, 96)
        for py in (0, 1):
            ap_t = np.zeros((3, 3, L, 128), f32)
            for n in range(3):
                u = py + 2 * n
                for m in range(3):
                    for half, v in ((0, 2 * m), (1, 2 * m + 1)):
                        ly_lo = max(0, (3 - u) // 2)
                        ly_hi = min(48, (99 - u) // 2)
                        lx_lo = max(0, (3 - v) // 2)
                        lx_hi = min(48, (97 - v) // 2 + 1)
                        Y0, X0 = 2 * ly_lo + u - 2, 2 * lx_lo + v - 2
                        blk = asm3[:, Y0:Y0 + 2 * (ly_hi - ly_lo):2,
                                   X0:X0 + 2 * (lx_hi - lx_lo):2]
                        dst = ap_t[n, m, :, 64 * half:64 * half + 64].reshape(48, 48, 64)
                        dst[ly_lo:ly_hi, lx_lo:lx_hi, :] = blk.transpose(1, 2, 0)
            # device ap layout: [p(128), k(18), nm(9), c(128)] for one-DMA load
            ap2 = np.ascontiguousarray(
                ap_t.reshape(9, NCH, 128, 128).transpose(2, 1, 0, 3)
            ).reshape(128, NCH * 9 * 128).astype(ml_dtypes.bfloat16)
            per_core.append({
                "qcolA": np.ascontiguousarray(q_col[:128]).astype(np.float16),
                "qcolB": np.ascontiguousarray(q_col[128:144]).astype(np.float16),
                "kpTA": np.ascontiguousarray(kpT[:128]).astype(np.float16),
                "kpTB": np.ascontiguousarray(kpT[128:144]).astype(np.float16),
                "ap": ap2,
            })
    return per_core


def kernel(x, wa, ba, aa, w1, b1, a1, w2, b2, a2):
    global last_exec_time_ns
    if "nc" not in _cache:
        _cache["nc"] = _build_program()
    nc = _cache["nc"]
    in_maps = _host_prep(np.asarray(x, np.float32), np.asarray(wa), np.asarray(ba),
                         np.asarray(aa), np.asarray(w1), np.asarray(b1),
                         np.asarray(a1), np.asarray(w2), np.asarray(b2),
                         np.asarray(a2))
    import os
    trace = bool(int(os.environ.get("KERNEL_TRACE", "0")))
    res = run_bass_kernel_spmd(nc, in_maps, core_ids=list(range(8)), trace=trace)
    last_exec_time_ns = res.exec_time_ns
    out = np.zeros((B, C, 192, 192), np.float32)
    for core in range(8):
        s, py = core // 2, core % 2
        r = res.results[core]["oh"].reshape(64, 96, 2, 96)
        out[s, :, py::2, 0::2] = r[:, :, 0, :]
        out[s, :, py::2, 1::2] = r[:, :, 1, :]
    return out


# revision 34
# speedup vs baseline: 10.9115x; 1.1830x over previous
"""CrossScaleAttention Trainium2 kernel: 8-core SPMD via bass/tile.

Sharding: core (s, py) = (core//2, core%2): sample s = core//2, output row
parity py. Each core computes full attention for its sample and the deconv
for its output-row parity. Host prepares small gather tensors (q_col, kpT,
ap taps — <0.1% of FLOPs); device does scores matmuls (fp16), softmax
(exp/Z/normalize, bf16 att) and the conv-transpose matmuls (bf16).

Schedule: double-buffered att stripes; the deconv matmuls of stripe s-1 are
interleaved into the ACT-bound softmax phase of stripe s so the PE never
idles waiting on exp.
"""
import sys, types
sys.path.insert(0, "/opt/trn_rl_repo")
import numpy as np
import ml_dtypes
from contextlib import ExitStack

# NTFF profile hook shim (image's antenv lacks axon_hooks)
try:
    import trn_agent_boot.trn_boot as _tb
    _hook = _tb._ntff_profile_via_ctypes('/opt/axon/libaxon_pjrt.so')
    _m = types.ModuleType("antenv.axon_hooks")
    _m.get_axon_ntff_profile_hook = lambda: _hook
    _m.set_axon_ntff_profile_hook = lambda h: None
    sys.modules["antenv.axon_hooks"] = _m
except Exception:
    pass

import concourse.bass as bass
import concourse.tile as tile
import concourse.mybir as mybir
from concourse import bacc
from concourse.bass_utils import run_bass_kernel_spmd

F32 = mybir.dt.float32
F32R = mybir.dt.float32r
F16 = mybir.dt.float16
BF16 = mybir.dt.bfloat16
AF = mybir.ActivationFunctionType

C, Cr, B, H, W, L = 64, 16, 4, 96, 96, 2304
NCH = 18           # l-chunks of 128
ST_A = 12          # a-rows (output row-pairs) per stripe
RWS = ST_A + 2     # att i-rows buffered per stripe
NST = 96 // ST_A   # stripes

last_exec_time_ns = None

_cache = {}


def _build_program():
    nc = bacc.Bacc("TRN2", target_bir_lowering=False, debug=False, num_devices=8)
    qA_d = nc.dram_tensor("qcolA", [72, H * W], F16, kind="ExternalInput").ap()
    qB_d = nc.dram_tensor("qcolB", [72, H * W], F16, kind="ExternalInput").ap()
    kA_d = nc.dram_tensor("kpTA", [72, L], F16, kind="ExternalInput").ap()
    kB_d = nc.dram_tensor("kpTB", [72, L], F16, kind="ExternalInput").ap()
    ap_d = nc.dram_tensor("ap", [128, NCH * 9 * 128], BF16, kind="ExternalInput").ap()
    oh_d = nc.dram_tensor("oh", [64, 96 * 192], F32, kind="ExternalOutput").ap()

    with tile.TileContext(nc) as tc:
        with ExitStack() as ctx:
            pm = ctx.enter_context(tc.tile_pool(name="main", bufs=1))
            pq = ctx.enter_context(tc.tile_pool(name="q", bufs=2))
            pob = ctx.enter_context(tc.tile_pool(name="ob", bufs=3))
            prz = ctx.enter_context(tc.tile_pool(name="rz", bufs=2))
            pps = ctx.enter_context(tc.tile_pool(name="ps", bufs=3, space="PSUM"))
            ppd = ctx.enter_context(tc.tile_pool(name="pd", bufs=3, space="PSUM"))
            ppz = ctx.enter_context(tc.tile_pool(name="pz", bufs=2, space="PSUM"))

            # persistent operands
            kA = pm.tile([72, L], F16, tag="kA")
            nc.sync.dma_start(kA[:], kA_d)
            kB = pm.tile([72, L], F16, tag="kB")
            nc.sync.dma_start(kB[:], kB_d)
            apall = pm.tile([128, NCH * 9 * 128], BF16, tag="apall")
            nc.sync.dma_start(apall[:], ap_d)
            # full [128,128] ones for Z: keeps the PE in full-array config and
            # leaves Z broadcast across all partitions (no separate bcast MM)
            o128 = pm.tile([128, 128], BF16, tag="o128")
            nc.vector.memset(o128[:], 1.0)

            # att stripe buffers (double-buffered), bf16, one big tile each:
            # layout per partition: [k(NCH), r(RWS), c(98)]; cols 0,97 are pad
            attb = []
            for h in range(2):
                t = pm.tile([128, NCH * RWS * 98], BF16, tag=f"att{h}")
                attb.append(t)

            def chunk_view(h, k):
                return attb[h][:, k * RWS * 98:(k + 1) * RWS * 98] \
                    .rearrange("p (r c) -> p r c", c=98)

            for h in range(2):
                for k in range(NCH):
                    v = chunk_view(h, k)
                    for pc in (0, 97):
                        nc.vector.memset(v[:, :, pc:pc + 1], 0.0)
                    # stripe-0 boundary row (i=-1) zero
                    nc.vector.memset(v[:, 0:1, :], 0.0)

            # deconv MM emitters: one group = 162 accumulating MMs over G a-rows
            # (k outer so the normalize->deconv dependency ramps one chunk at
            # a time instead of needing 9 chunks normalized up front)
            def deconv_mms(h, g0, G):
                """Operand list for the 162 matmuls of a G-a-row deconv group."""
                out = []
                for k in range(NCH):
                    v = chunk_view(h, k)
                    for n in range(3):
                        for m in range(3):
                            nm = n * 3 + m
                            r0 = g0 + 2 - n
                            off = (k * 9 + nm) * 128
                            rhs = v[:, r0:r0 + G, 2 - m:98 - m]
                            out.append((apall[:, off:off + 128], rhs))
                return out

            # state of the pending (previous-stripe) deconv
            pending = None   # (h, arow, G, mm list, next index, dps tile)
            deferred = None  # deferred normalize tail of the previous group

            def emit_deconv_slice(cnt):
                nonlocal pending
                while cnt > 0:
                    if pending is None:
                        if not deconv_queue:
                            return
                        start_deconv(*deconv_queue.pop(0))
                    h, arow, G, mms, idx, dps = pending
                    end = min(idx + cnt, len(mms))
                    for i in range(idx, end):
                        lw, rhs = mms[i]
                        nc.tensor.matmul(dps[:, :G * 96], lw, rhs,
                                         start=(i == 0), stop=(i == len(mms) - 1))
                    cnt -= end - idx
                    if end == len(mms):
                        ob = pob.tile([128, 480], F32, tag="ob")
                        nc.scalar.activation(ob[:, :G * 96], dps[:, :G * 96], AF.Copy)
                        oap = oh_d.rearrange("p (y x) -> p y x", x=192)
                        nc.sync.dma_start(oap[:, arow:arow + G, 0:96],
                                          ob[0:64, :G * 96].rearrange("p (r c) -> p r c", c=96))
                        nc.sync.dma_start(oap[:, arow:arow + G, 96:192],
                                          ob[64:128, :G * 96].rearrange("p (r c) -> p r c", c=96))
                        pending = None
                    else:
                        pending = (h, arow, G, mms, end, dps)

            def start_deconv(h, arow, g0, G):
                nonlocal pending
                assert pending is None
                dps = ppd.tile([128, 480], F32, tag="dps")
                pending = (h, arow, G, deconv_mms(h, g0, G), 0, dps)

            deconv_queue = []   # (h, arow, g0, G) groups not yet started

            for st in range(NST):
                h = st % 2
                a0 = st * ST_A
                r_lo = 1 if st == 0 else 0
                r_hi = RWS - 1 if st == NST - 1 else RWS
                i_lo = a0 - 1 + r_lo
                nrows = r_hi - r_lo
                qA = pq.tile([72, RWS * 96], F16, tag="qA")
                qB = pq.tile([72, RWS * 96], F16, tag="qB")
                nc.sync.dma_start(qA[:, r_lo * 96: (r_lo + nrows) * 96],
                                  qA_d[:, i_lo * 96: (i_lo + nrows) * 96])
                nc.sync.dma_start(qB[:, r_lo * 96: (r_lo + nrows) * 96],
                                  qB_d[:, i_lo * 96: (i_lo + nrows) * 96])
                if st == NST - 1:
                    # boundary row (i=96) zero, this buffer's last row
                    for k in range(NCH):
                        nc.vector.memset(chunk_view(h, k)[:, RWS - 1:RWS, :], 0.0)

                groups = []
                r = r_lo
                while r < r_hi:
                    sz = min(5, r_hi - r)
                    groups.append((r, sz))
                    r += sz
                for (rg, sz) in groups:
                    N = sz * 96
                    zps = ppz.tile([128, 512], F32, tag="zps")
                    for k in range(NCH):
                        ps = pps.tile([128, 512], F32, tag="ps")
                        nc.tensor.matmul(ps[:, :N], kA[:, 128 * k:128 * (k + 1)],
                                         qA[:, rg * 96: rg * 96 + N],
                                         start=True, stop=False)
                        nc.tensor.matmul(ps[:, :N], kB[:, 128 * k:128 * (k + 1)],
                                         qB[:, rg * 96: rg * 96 + N],
                                         start=False, stop=True)
                        # fill PE with previous-stripe deconv while ACT exps
                        emit_deconv_slice(9)
                        if k == 3 and deferred is not None:
                            deferred()
                            deferred = None
                        # exp(s) from psum -> att rows (scale folded into kpT)
                        dst = chunk_view(h, k)[:, rg:rg + sz, 1:97]
                        nc.scalar.activation(dst, ps[:, :N].rearrange("p (r c) -> p r c", c=96),
                                             AF.Exp)
                        # Z accumulation, pre-broadcast to all 128 partitions
                        nc.tensor.matmul(zps[:, :N], o128[:], dst,
                                         start=(k == 0), stop=(k == NCH - 1))

                    def tail(h=h, rg=rg, sz=sz, N=N, zps=zps):
                        # normalize: 1/Z straight to bf16, then scale att
                        bsb = prz.tile([128, 512], BF16, tag="bsb")
                        with nc.allow_low_precision(reason="1/Z in bf16 scales att"):
                            nc.vector.reciprocal(bsb[:, :N], zps[:, :N])
                        for k in range(NCH):
                            a_ap = chunk_view(h, k)[:, rg:rg + sz, 1:97]
                            nc.vector.tensor_mul(a_ap, a_ap,
                                                 bsb[:, :N].rearrange("p (r c) -> p r c", c=96))
                    deferred = tail

                # queue this stripe's deconv groups (run during next stripe)
                g0 = 0
                while g0 < ST_A:
                    G = min(5, ST_A - g0)
                    deconv_queue.append((h, a0 + g0, g0, G))
                    g0 += G
                # drain any unfinished pending deconv before stripe ends?
                # no — let it continue into the next stripe's blocks.

            # flush the last normalize tail, then drain remaining deconv groups
            if deferred is not None:
                deferred()
                deferred = None
            emit_deconv_slice(10 ** 9)
    nc.compile()
    return nc


def _prelu(z, a):
    return np.where(z >= 0, z, a * z)


def _host_prep(x, wa, ba, aa, w1, b1, a1, w2, b2, a2):
    """Per-sample gather prep (numpy, validated vs reference)."""
    f32 = np.float32
    per_core = []
    waT_aug = (np.concatenate([wa.T, ba[None, :]], 0) / 6.0).astype(f32)
    w1T_aug = np.concatenate([w1.T, b1[None, :]], 0).astype(f32)
    w2T_aug = np.concatenate([w2.T / 4.0, b2[None, :]], 0).astype(f32)
    aav, a1v, a2v = float(aa[0]), float(a1[0]), float(a2[0])
    for s in range(B):
        xs = np.asarray(x[s], f32)
        xq_aug = np.concatenate([xs.reshape(64, -1), np.ones((1, H * W), f32)], 0)
        asmT = _prelu(xq_aug.T @ waT_aug, aav)
        qT = _prelu(xq_aug.T @ w1T_aug, a1v)
        x3 = xs.reshape(64, 96, 96)
        t1 = x3[:, :, 0::2] + x3[:, :, 1::2]
        xd = t1[:, 0::2, :] + t1[:, 1::2, :]
        xd_aug = np.concatenate([xd.reshape(64, -1), np.ones((1, 48 * 48), f32)], 0)
        kfT = _prelu(xd_aug.T @ w2T_aug, a2v)

        kf = kfT.T.reshape(Cr, 48, 48)
        kpT = np.zeros((144, L), f32)
        for t, (dy, dx) in enumerate([(a, b) for a in range(3) for b in range(3)]):
            ly_lo, ly_hi = max(0, 1 - dy), min(48, 49 - dy)
            lx_lo, lx_hi = max(0, 1 - dx), min(48, 49 - dx)
            blk = kf[:, ly_lo + dy - 1:ly_hi + dy - 1, lx_lo + dx - 1:lx_hi + dx - 1]
            dst = kpT[16 * t:16 * t + 16].reshape(Cr, 48, 48)
            dst[:, ly_lo:ly_hi, lx_lo:lx_hi] = blk
        nrm = np.sqrt((kpT ** 2).sum(0))
        rnorm10 = (10.0 / np.maximum(nrm, 1e-4)).astype(f32)
        # fold the softmax scale / norm into kpT: scores psum = 10*s/norm
        kpT = kpT * rnorm10[None, :]

        q3 = qT.T.reshape(Cr, 96, 96)
        q_col = np.zeros((144, 96, 96), f32)
        for t, (dy, dx) in enumerate([(a, b) for a in range(3) for b in range(3)]):
            y_lo, y_hi = max(0, 1 - dy), min(96, 97 - dy)
            x_lo, x_hi = max(0, 1 - dx), min(96, 97 - dx)
            q_col[16 * t:16 * t + 16, y_lo:y_hi, x_lo:x_hi] = \
                q3[:, y_lo + dy - 1:y_hi + dy - 1, x_lo + dx - 1:x_hi + dx - 1]
        q_col = q_col.reshape(144, H * W)

        asm3 = asmT.T.reshape(64, 96, 96)
        for py in (0, 1):
            ap_t = np.zeros((3, 3, L, 128), f32)
            for n in range(3):
                u = py + 2 * n
                for m in range(3):
                    for half, v in ((0, 2 * m), (1, 2 * m + 1)):
                        ly_lo = max(0, (3 - u) // 2)
                        ly_hi = min(48, (99 - u) // 2)
                        lx_lo = max(0, (3 - v) // 2)
                        lx_hi = min(48, (97 - v) // 2 + 1)
                        Y0, X0 = 2 * ly_lo + u - 2, 2 * lx_lo + v - 2
                        blk = asm3[:, Y0:Y0 + 2 * (ly_hi - ly_lo):2,
                                   X0:X0 + 2 * (lx_hi - lx_lo):2]
                        dst = ap_t[n, m, :, 64 * half:64 * half + 64].reshape(48, 48, 64)
                        dst[ly_lo:ly_hi, lx_lo:lx_hi, :] = blk.transpose(1, 2, 0)
            # device ap layout: [p(128), k(18), nm(9), c(128)] for one-DMA load
            ap2 = np.ascontiguousarray(
                ap_t.reshape(9, NCH, 128, 128).transpose(2, 1, 0, 3)
            ).reshape(128, NCH * 9 * 128).astype(ml_dtypes.bfloat16)
            per_core.append({
                "qcolA": np.ascontiguousarray(q_col[:72]).astype(np.float16),
                "qcolB": np.ascontiguousarray(q_col[72:144]).astype(np.float16),
                "kpTA": np.ascontiguousarray(kpT[:72]).astype(np.float16),
                "kpTB": np.ascontiguousarray(kpT[72:144]).astype(np.float16),
                "ap": ap2,
            })
    return per_core


def kernel(x, wa, ba, aa, w1, b1, a1, w2, b2, a2):
    global last_exec_time_ns
    if "nc" not in _cache:
        _cache["nc"] = _build_program()
    nc = _cache["nc"]
    in_maps = _host_prep(np.asarray(x, np.float32), np.asarray(wa), np.asarray(ba),
                         np.asarray(aa), np.asarray(w1), np.asarray(b1),
                         np.asarray(a1), np.asarray(w2), np.asarray(b2),
                         np.asarray(a2))
    import os
    trace = bool(int(os.environ.get("KERNEL_TRACE", "0")))
    res = run_bass_kernel_spmd(nc, in_maps, core_ids=list(range(8)), trace=trace)
    last_exec_time_ns = res.exec_time_ns
    out = np.zeros((B, C, 192, 192), np.float32)
    for core in range(8):
        s, py = core // 2, core % 2
        r = res.results[core]["oh"].reshape(64, 96, 2, 96)
        out[s, :, py::2, 0::2] = r[:, :, 0, :]
        out[s, :, py::2, 1::2] = r[:, :, 1, :]
    return out
